# revision 37
# baseline (speedup 1.0000x reference)
"""Trainium2 Bass kernel for nn_LossSupervisedTags (tag + heatmap MSE loss).

Contract: kernel(**inputs) takes the FULL unsharded inputs (as produced by
setup_inputs) and returns the FULL scalar output.  Internally the batch dim
(B=32) is sharded 4-images-per-core across 8 NeuronCores; each core computes
its local tag / heatmap loss partial sums on device, and the host combines
the 8 partial sums into the final scalar mean.

Host staging: slices per-core shards, transposes dets/heat to [h, p, w] so
every DMA descriptor is a fat contiguous run, and gathers the 510 predicted
tags per image (index staging for the tag loss).

Per-core device pipeline (per image b, stacks s=0..3):
  DVE   : diff = dets[b,s] - heat[b]      (layout [h=128, (p,w)=2176], bf16 2x)
  ACT/DVE: sq  = diff^2 into sq_img[:, s*2176:...] as bf16
  PE    : 32 matmuls per image, each covering a group of 4 w-columns:
          lhsT = mask[:, 4g:4g+4] (bf16), rhs = sq viewed as [h, w4, s, p].
          Group g targets PE column quadrant g%4 (tile_position) so
          consecutive LDWEIGHTS+MATMUL pairs pipeline in different
          sub-arrays.  Each image accumulates into its OWN PSUM bank;
          right after image b's last matmul the bank is copied (DVE) into
          acc[:, b*272:...] so only image 3's chain sits in the tail.
          Image 3 runs one matmul round PER STACK (rhs 68 cols) so the
          final round starts as soon as the last stack's sq is ready.
  tag   : gathered pred tags packed [128,64]; (pt-gt)^2*vis summed on the
          POOL engine (gpsimd) so the slow scalar-ring tagin transfer never
          blocks the DVE pipeline.

DMA plan: 11 transfers on the sync HWDGE ring in need-order, one tile per
transfer (multiple outstanding DMAs into one tile serialize on its
semaphore).  mask/tagin and the tag output ride the scalar HWDGE ring;
descriptors within a ring drain FIFO.
"""

import sys
import types

import ml_dtypes
import numpy as np

import concourse.bacc as bacc
import concourse.mybir as mybir
from concourse.tile import TileContext
from concourse.bass_utils import run_bass_kernel_spmd

# If BASS_TRACE is set in the environment but this image lacks
# antenv.axon_hooks, run_bass_kernel_spmd would die on import; register a
# no-op hook module so tracing degrades gracefully instead.
try:
    import antenv.axon_hooks  # noqa: F401
except ImportError:
    try:
        import antenv

        _m = types.ModuleType("antenv.axon_hooks")
        _m.get_axon_ntff_profile_hook = lambda: None
        _m.set_axon_ntff_profile_hook = lambda h: None
        sys.modules["antenv.axon_hooks"] = _m
        antenv.axon_hooks = _m
    except ImportError:
        pass

# Problem constants (hardcoded per harness contract)
B, S, C, H, W = 32, 4, 34, 128, 128
N_PARTS, TAG_DIM, M = 17, 1, 30
TAG_W, HM_W = 0.001, 1.0
NCORES = 8
BLOC = B // NCORES            # 4 images per core
FREE = N_PARTS * W            # 2176 free elems per (b, s) tile
KP = M * N_PARTS              # 510 keypoints per image
KP_COLS = 4                   # ceil(510 / 128) columns per (b, s)
TAG_COLS = BLOC * S * KP_COLS  # 64
WG = 8                        # w-columns per matmul group
NG = W // WG                  # 16 matmul groups per round
RSQ = WG * 2 * N_PARTS        # 272 psum cols per (image, stack-pair) region
PSTRIDE = 512                 # psum region stride (bank-aligned, f32 elems)
OUTW = PSTRIDE + RSQ          # 784 cols copied out per image (incl. gap)

_cache = {}

# Elementwise plan.  Subs all ride DVE (only engine with 2-tensor bf16 2x)
# in pure arrival order — anything interleaved into DVE's queue delays later
# subs through the scheduler's quantized cross-engine waits.  ACT owns all
# six pair squares (kept ~75% duty, arrival-paced gaps dodge the DMA
# throttle); image 3's four squares land on DVE right after its subs (the
# stream is done by then, DVE is free, and the 1.3us/square latency beats
# ACT's 2.1 in the tail).


def _build():
    f32 = mybir.dt.float32
    bf16 = mybir.dt.bfloat16
    nc = bacc.Bacc(
        "TRN2", target_bir_lowering=False, debug=False, num_devices=NCORES
    )
    # Host pre-interleaves dets into 2-stack pairs [pair, H, (s p w)] and
    # heat into 2-image pairs [pair, H, (b p w)] so every bulk transfer is a
    # single contiguous 8.7KB run per partition (max DMA-engine efficiency);
    # image 3's last two stacks stay as single-stack transfers [H, (p w)]
    # for fine-grained tail chasing.
    dets_p = nc.dram_tensor(
        "dets_p", [7, H, 2 * FREE], bf16, kind="ExternalInput"
    )
    dets_s = nc.dram_tensor("dets_s", [2, H, FREE], bf16, kind="ExternalInput")
    heat_p = nc.dram_tensor("heat_p", [2, H, 2 * FREE], bf16, kind="ExternalInput")
    maskw = nc.dram_tensor("maskw", [H, BLOC * W], bf16, kind="ExternalInput")
    tagin = nc.dram_tensor("tagin", [128, 3 * TAG_COLS], f32, kind="ExternalInput")
    out_det = nc.dram_tensor(
        "out_det", [128, BLOC * 2 * 2 * WG], f32, kind="ExternalOutput"
    )
    out_tag = nc.dram_tensor("out_tag", [1, 1], f32, kind="ExternalOutput")

    with TileContext(nc) as tc:
        with (
            tc.tile_pool(name="const", bufs=1) as cpool,
            tc.tile_pool(name="heatp", bufs=2) as hpool,
            tc.tile_pool(name="detp", bufs=7) as dpool,
            tc.tile_pool(name="dets3", bufs=2) as spool,
            tc.tile_pool(name="diffp", bufs=3) as fpool,
            tc.tile_pool(name="diffs", bufs=2) as spool_f,
            tc.tile_pool(name="sqp", bufs=4) as qpool,
            tc.tile_pool(name="psum0", bufs=1, space="PSUM") as ppool0,
            tc.tile_pool(name="psum1", bufs=1, space="PSUM") as ppool1,
            tc.tile_pool(name="psum2", bufs=1, space="PSUM") as ppool2,
            tc.tile_pool(name="psum3", bufs=1, space="PSUM") as ppool3,
        ):
            # Small inputs ride the scalar HWDGE ring so they don't queue
            # behind the bulk det stream on the sync ring.
            mask_t = cpool.tile([128, BLOC * W], bf16)
            nc.scalar.dma_start(out=mask_t[:], in_=maskw[:])
            tag_t = cpool.tile([128, 3 * TAG_COLS], f32)
            nc.scalar.dma_start(out=tag_t[:], in_=tagin[:])

            # Dummy activation so the Square table set loads during the DMA
            # ramp instead of delaying the first real square.
            warm_t = cpool.tile([1, 8], f32)
            warm_o = cpool.tile([1, 8], f32)
            nc.gpsimd.memset(warm_t[:], 0.0)
            nc.scalar.activation(
                warm_o[:], warm_t[:], mybir.ActivationFunctionType.Square
            )

            # 11 transfers up-front on the sync ring in need-order, one tile
            # per transfer.  Measured facts behind this shape: (a) single
            # contiguous runs per partition are the efficient DMA unit (the
            # host interleave makes pairs one 8.7KB run); (b) multiple
            # outstanding DMAs into ONE tile serialize on that tile's
            # semaphore; (c) the dynamic-HWDGE path keeps ~12 transfers in
            # flight and stalls the issuing sequencer when full — slots free
            # progressively, so the last two (needed last) still arrive in
            # time.  Within a ring, descriptors drain FIFO.
            # Issue order: image 0/1 pairs, heat1, then image 3's two singles
            # MID-stream (their chains fill DVE's natural gaps, and the
            # stream tail — where the activity throttle may clip DMA — then
            # carries pairs that are mostly landed before it fires), then
            # image 2's pairs and image 3's s01 pair last.
            heat_tiles = {}
            det_tiles = {}
            pair_tiles = {}

            def emit_heat(b0):
                hp = hpool.tile([128, 2 * FREE], bf16, name="heat_t", tag="heat_t")
                nc.sync.dma_start(out=hp[:], in_=heat_p[b0 // 2])
                heat_tiles[b0] = hp[:, 0:FREE]
                heat_tiles[b0 + 1] = hp[:, FREE : 2 * FREE]

            def emit_pair(b, s0, pair_idx):
                dp = dpool.tile([128, 2 * FREE], bf16, name="det_t", tag="det_t")
                nc.sync.dma_start(out=dp[:], in_=dets_p[pair_idx])
                pair_tiles[(b, s0)] = dp[:]
                det_tiles[(b, s0)] = dp[:, 0:FREE]
                det_tiles[(b, s0 + 1)] = dp[:, FREE : 2 * FREE]

            emit_heat(0)
            emit_pair(0, 0, 0)
            emit_pair(0, 2, 1)
            emit_pair(1, 0, 2)
            emit_pair(1, 2, 3)
            emit_heat(2)
            for s in (2, 3):
                ds = spool.tile([128, FREE], bf16, name="det_s", tag="det_s")
                nc.sync.dma_start(out=ds[:], in_=dets_s[s - 2])
                det_tiles[(BLOC - 1, s)] = ds[:]
            emit_pair(2, 0, 4)
            emit_pair(2, 2, 5)
            emit_pair(3, 0, 6)

            acc_det = cpool.tile([128, BLOC * 2 * 2 * WG], f32)

            def tag_block():
                # Tag loss (tiny) on the POOL engine: gated only by the
                # slow scalar-ring tagin transfer, which Pool can wait on
                # without holding up the DVE det pipeline.  The out_tag DMA
                # issues from Pool (SWDGE) so its tag-semaphore wait can
                # never park the ACT or sync sequencers.
                ptg_t = tag_t[:, 0:TAG_COLS]
                gtv_t = tag_t[:, TAG_COLS : 2 * TAG_COLS]
                vis_t = tag_t[:, 2 * TAG_COLS : 3 * TAG_COLS]
                e_t = cpool.tile([128, TAG_COLS], f32)
                ev_t = cpool.tile([128, TAG_COLS], f32)
                scr_t = cpool.tile([128, TAG_COLS], f32)
                tag_acc = cpool.tile([1, 1], f32)
                nc.gpsimd.tensor_sub(e_t[:], ptg_t, gtv_t)
                nc.gpsimd.tensor_mul(ev_t[:], e_t[:], vis_t)
                nc.gpsimd.tensor_mul(scr_t[:], e_t[:], ev_t[:])
                nc.gpsimd.tensor_reduce(
                    tag_acc[:], scr_t[:],
                    axis=mybir.AxisListType.XYZWC, op=mybir.AluOpType.add,
                )
                nc.gpsimd.dma_start(out=out_tag[:], in_=tag_acc[:])

            # ---- heatmap (det) loss ----
            # Each image owns a 2-bank [128, 1024] f32 PSUM tile: the s01
            # round accumulates in region [0:272] (bank 0), s23 in [512:784]
            # (bank 1) — per-matmul accumulation never crosses a 2KB bank.
            # 16 matmuls per round, each covering 8 w-columns: lhsT = mask
            # 8-col slice, rhs = sq viewed [h, w8, s, p].  Group g targets
            # PE column quadrant g % 4 (tile_position) so consecutive
            # LDWEIGHTS+MATMUL pairs pipeline in different sub-arrays; the
            # 4 groups of a quadrant accumulate.  Useful outputs:
            # psum[32q+j, region + j-th 34-block].
            def emit_round(b, psum_t, sq_g, s0, s1):
                region = 0 if s0 < 2 else PSTRIDE
                psum_r = psum_t[:, region : region + RSQ].rearrange(
                    "m (j s p) -> m j s p", j=WG, s=2
                )
                srel0, srel1 = s0 % 2, (s1 - 1) % 2 + 1
                for g in range(NG):
                    q = g % 4
                    nc.tensor.matmul(
                        psum_r[32 * q : 32 * q + WG, :, srel0:srel1, :],
                        lhsT=mask_t[:, b * W + WG * g : b * W + WG * (g + 1)],
                        rhs=sq_g[:, WG * g : WG * (g + 1), s0:s1, :],
                        start=(g < 4),
                        stop=(g >= NG - 4),
                        tile_position=(0, 32 * q),
                    )

            ppools = [ppool0, ppool1, ppool2, ppool3]
            psum_ts = {}
            sq_ts = {}
            sq_gs = {}

            def img_state(b):
                sq_t = qpool.tile([128, S * FREE], bf16, name=f"sq{b}", tag="sq")
                psum_t = ppools[b].tile(
                    [128, 2 * PSTRIDE], f32, name=f"ps{b}", tag=f"ps{b}"
                )
                psum_ts[b] = psum_t
                sq_ts[b] = sq_t
                sq_gs[b] = sq_t[:].rearrange(
                    "q (s p w) -> q w s p", s=S, p=N_PARTS
                )

            def pair_unit(b, s0, sq_eng="act", round_=True):
                # one sub + one square per 2-stack slab (heat broadcast)
                heat_b = heat_tiles[b].unsqueeze(1).broadcast_to([128, 2, FREE])
                diff_t = fpool.tile(
                    [128, 2 * FREE], bf16, name="diffp_t", tag="diffp_t"
                )
                src3 = pair_tiles[(b, s0)].rearrange("p (s f) -> p s f", s=2)
                dst3 = diff_t[:].rearrange("p (s f) -> p s f", s=2)
                nc.vector.tensor_sub(dst3, src3, heat_b)
                sqdst = sq_ts[b][:, s0 * FREE : (s0 + 2) * FREE]
                if sq_eng == "act":
                    nc.scalar.activation(
                        sqdst, diff_t[:], mybir.ActivationFunctionType.Square
                    )
                else:
                    nc.vector.tensor_mul(sqdst, diff_t[:], diff_t[:])
                if round_:
                    emit_round(b, psum_ts[b], sq_gs[b], s0, s0 + 2)

            def single_sub(b, s):
                diff_t = spool_f.tile(
                    [128, FREE], bf16, name="diffs_t", tag="diffs_t"
                )
                nc.vector.tensor_sub(diff_t[:], det_tiles[(b, s)], heat_tiles[b])
                return diff_t

            def single_sq(b, s, diff_t):
                dst = sq_ts[b][:, s * FREE : (s + 1) * FREE]
                nc.vector.tensor_mul(dst, diff_t[:], diff_t[:])
                emit_round(b, psum_ts[b], sq_gs[b], s, s + 1)

            for b in range(BLOC):
                img_state(b)
            # Images 0-1: pairs, ACT squares, rounds inline.
            pair_unit(0, 0)
            pair_unit(0, 2)
            tag_block()
            pair_unit(1, 0)
            pair_unit(1, 2)
            # Image 3's s2/s3 arrive mid-stream; their subs run in DVE's
            # arrival gaps, but image 2's s01 sub (data lands between them)
            # is emitted BEFORE their squares so DVE consumes strictly in
            # arrival order.
            d32 = single_sub(3, 2)
            d33 = single_sub(3, 3)
            pair_unit(2, 0, round_=False)  # sub + ACT square; round below
            single_sq(3, 2, d32)
            single_sq(3, 3, d33)
            emit_round(2, psum_ts[2], sq_gs[2], 0, 2)
            # Image 2 s23: ACT square; its round is deferred behind a
            # virtual-time fence so the static scheduler cannot hoist it
            # (nor the late reduces) into DVE/PE ahead of data-gated work.
            pair_unit(2, 2, round_=False)
            # Image 3's s01 pair closes the stream: sub + square on DVE.
            pair_unit(3, 0, sq_eng="dve")
            emit_round(2, psum_ts[2], sq_gs[2], 2, 4)

            # Per-region p-axis reductions (DVE, ~0.45us each) emitted LAST:
            # each waits on its region's final matmul; the out DMA then
            # carries only [128, 128] f32.  Latest-dependency regions go
            # last so the static scheduler cannot park DVE on them.
            def reduce_region(b, region, off):
                src = psum_ts[b][:, region * PSTRIDE : region * PSTRIDE + RSQ]
                dst = acc_det[:, off : off + 2 * WG].rearrange(
                    "m (j s) -> m j s", j=WG
                )
                nc.vector.reduce_sum(
                    dst, src.rearrange("m (j s p) -> m j s p", j=WG, s=2),
                    axis=mybir.AxisListType.X,
                )

            order = [(0, 0), (0, 1), (1, 0), (1, 1), (3, 1), (2, 0), (3, 0), (2, 1)]
            for b, region in order:
                reduce_region(b, region, (b * 2 + region) * 2 * WG)

            # Output DMA issues from the sync sequencer, which is parked on
            # its wait by then and fires in parallel with the copy.
            nc.sync.dma_start(out=out_det[:], in_=acc_det[:])
    nc.compile()
    return nc


def _pack(vals):
    """vals: (BLOC, S, KP) float32 -> [128, TAG_COLS] with col = b*16+s*4+j,
    partition k holding element j*128+k of the zero-padded 512 vector."""
    padded = np.zeros((BLOC, S, KP_COLS * 128), np.float32)
    padded[..., :KP] = vals
    return (
        padded.reshape(BLOC, S, KP_COLS, 128)
        .transpose(3, 0, 1, 2)
        .reshape(128, TAG_COLS)
    )


def kernel(preds, masks, keypoints_idx, keypoints_vis, gt_tags, heatmaps):
    preds = np.asarray(preds, dtype=np.float32)
    masks = np.asarray(masks, dtype=np.float32)
    keypoints_idx = np.asarray(keypoints_idx)
    keypoints_vis = np.asarray(keypoints_vis, dtype=np.float32)
    gt_tags = np.asarray(gt_tags, dtype=np.float32)
    heatmaps = np.asarray(heatmaps, dtype=np.float32)

    if "nc" not in _cache:
        _cache["nc"] = _build()
    nc = _cache["nc"]

    # Host-side input staging: gather predicted tags at keypoint locations
    # (index-based staging; all loss arithmetic runs on device).
    tags = preds[:, :, N_PARTS:].reshape(B, S, N_PARTS * H * W)
    flat_idx = keypoints_idx.reshape(B, 1, KP).astype(np.int64)
    pt = np.take_along_axis(tags, np.broadcast_to(flat_idx, (B, S, KP)), axis=2)
    gt = gt_tags.reshape(B, KP)
    vi = keypoints_vis.reshape(B, KP)

    in_maps = []
    for c in range(NCORES):
        b0 = c * BLOC
        sl = slice(b0, b0 + BLOC)
        tag_in = np.concatenate(
            [
                _pack(pt[sl]),
                _pack(np.broadcast_to(gt[sl][:, None, :], (BLOC, S, KP))),
                _pack(np.broadcast_to(vi[sl][:, None, :], (BLOC, S, KP))),
            ],
            axis=1,
        )
        # [BLOC, S, 17, H, W] -> [BLOC, S, H, 17, W], then pack 2-stack
        # pairs as [H, (s p w)] and heat 2-image pairs as [H, (b p w)] so
        # each bulk DMA is one contiguous run per partition.
        dt_ = preds[sl, :, :N_PARTS].transpose(0, 1, 3, 2, 4)
        ht_ = heatmaps[sl].transpose(0, 2, 1, 3)
        dets_p = np.stack(
            [
                dt_[b, s0 : s0 + 2].transpose(1, 0, 2, 3).reshape(H, 2 * FREE)
                for b, s0 in (
                    (0, 0), (0, 2), (1, 0), (1, 2), (2, 0), (2, 2), (3, 0),
                )
            ]
        )
        dets_s = dt_[3, 2:4].reshape(2, H, FREE)
        heat_pk = np.stack(
            [
                ht_[b0 : b0 + 2].transpose(1, 0, 2, 3).reshape(H, 2 * FREE)
                for b0 in (0, 2)
            ]
        )
        in_maps.append(
            {
                "dets_p": np.ascontiguousarray(dets_p).astype(ml_dtypes.bfloat16),
                "dets_s": np.ascontiguousarray(dets_s).astype(ml_dtypes.bfloat16),
                "heat_p": np.ascontiguousarray(heat_pk).astype(ml_dtypes.bfloat16),
                # [BLOC, H, W] -> [H, BLOC*W]
                "maskw": np.ascontiguousarray(
                    masks[sl].transpose(1, 0, 2).reshape(H, BLOC * W)
                ).astype(ml_dtypes.bfloat16),
                "tagin": np.ascontiguousarray(tag_in),
            }
        )

    res = run_bass_kernel_spmd(nc, in_maps, list(range(NCORES)))
    _cache["last_results"] = res

    det_total = 0.0
    tag_total = 0.0
    for r in res.results:
        od = r["out_det"].astype(np.float64)
        # image b, region r2 (stack pair): row 32q+i, cols (j, s) with the
        # diag j==i holding the masked partial sums (p pre-reduced on DVE)
        for bi in range(BLOC):
            for r2 in range(2):
                for q in range(4):
                    for i in range(WG):
                        c0 = (bi * 2 + r2) * 2 * WG + i * 2
                        det_total += float(od[32 * q + i, c0 : c0 + 2].sum())
        tag_total += float(r["out_tag"].sum(dtype=np.float64))

    det_mean = det_total / (B * S * N_PARTS * H * W)
    tag_mean = tag_total / (B * S)
    return np.float32(TAG_W * tag_mean + HM_W * det_mean)


# revision 38
# speedup vs baseline: 1.0339x; 1.0339x over previous
"""Trainium2 Bass kernel for nn_LossSupervisedTags (tag + heatmap MSE loss).

Contract: kernel(**inputs) takes the FULL unsharded inputs (as produced by
setup_inputs) and returns the FULL scalar output.  Internally the batch dim
(B=32) is sharded 4-images-per-core across 8 NeuronCores; each core computes
its local tag / heatmap loss partial sums on device, and the host combines
the 8 partial sums into the final scalar mean.

Host staging: slices per-core shards, transposes dets/heat to [h, p, w] so
every DMA descriptor is a fat contiguous run, and gathers the 510 predicted
tags per image (index staging for the tag loss).

Per-core device pipeline (per image b, stacks s=0..3):
  DVE   : diff = dets[b,s] - heat[b]      (layout [h=128, (p,w)=2176], bf16 2x)
  ACT/DVE: sq  = diff^2 into sq_img[:, s*2176:...] as bf16
  PE    : 32 matmuls per image, each covering a group of 4 w-columns:
          lhsT = mask[:, 4g:4g+4] (bf16), rhs = sq viewed as [h, w4, s, p].
          Group g targets PE column quadrant g%4 (tile_position) so
          consecutive LDWEIGHTS+MATMUL pairs pipeline in different
          sub-arrays.  Each image accumulates into its OWN PSUM bank;
          right after image b's last matmul the bank is copied (DVE) into
          acc[:, b*272:...] so only image 3's chain sits in the tail.
          Image 3 runs one matmul round PER STACK (rhs 68 cols) so the
          final round starts as soon as the last stack's sq is ready.
  tag   : gathered pred tags packed [128,64]; (pt-gt)^2*vis summed on the
          POOL engine (gpsimd) so the slow scalar-ring tagin transfer never
          blocks the DVE pipeline.

DMA plan: 11 transfers on the sync HWDGE ring in need-order, one tile per
transfer (multiple outstanding DMAs into one tile serialize on its
semaphore).  mask/tagin and the tag output ride the scalar HWDGE ring;
descriptors within a ring drain FIFO.
"""

import sys
import types

import ml_dtypes
import numpy as np

import concourse.bacc as bacc
import concourse.mybir as mybir
from concourse.tile import TileContext
from concourse.bass_utils import run_bass_kernel_spmd

# If BASS_TRACE is set in the environment but this image lacks
# antenv.axon_hooks, run_bass_kernel_spmd would die on import; register a
# no-op hook module so tracing degrades gracefully instead.
try:
    import antenv.axon_hooks  # noqa: F401
except ImportError:
    try:
        import antenv

        _m = types.ModuleType("antenv.axon_hooks")
        _m.get_axon_ntff_profile_hook = lambda: None
        _m.set_axon_ntff_profile_hook = lambda h: None
        sys.modules["antenv.axon_hooks"] = _m
        antenv.axon_hooks = _m
    except ImportError:
        pass

# Problem constants (hardcoded per harness contract)
B, S, C, H, W = 32, 4, 34, 128, 128
N_PARTS, TAG_DIM, M = 17, 1, 30
TAG_W, HM_W = 0.001, 1.0
NCORES = 8
BLOC = B // NCORES            # 4 images per core
FREE = N_PARTS * W            # 2176 free elems per (b, s) tile
KP = M * N_PARTS              # 510 keypoints per image
KP_COLS = 4                   # ceil(510 / 128) columns per (b, s)
TAG_COLS = BLOC * S * KP_COLS  # 64
WG = 8                        # w-columns per matmul group
NG = W // WG                  # 16 matmul groups per round
RSQ = WG * 2 * N_PARTS        # 272 psum cols per (image, stack-pair) region
PSTRIDE = 512                 # psum region stride (bank-aligned, f32 elems)
OUTW = PSTRIDE + RSQ          # 784 cols copied out per image (incl. gap)

_cache = {}

# Elementwise plan.  Subs all ride DVE (only engine with 2-tensor bf16 2x)
# in pure arrival order — anything interleaved into DVE's queue delays later
# subs through the scheduler's quantized cross-engine waits.  ACT owns all
# six pair squares (kept ~75% duty, arrival-paced gaps dodge the DMA
# throttle); image 3's four squares land on DVE right after its subs (the
# stream is done by then, DVE is free, and the 1.3us/square latency beats
# ACT's 2.1 in the tail).


def _build():
    f32 = mybir.dt.float32
    bf16 = mybir.dt.bfloat16
    nc = bacc.Bacc(
        "TRN2", target_bir_lowering=False, debug=False, num_devices=NCORES
    )
    # Host pre-interleaves dets into 2-stack pairs [pair, H, (s p w)] and
    # heat into 2-image pairs [pair, H, (b p w)] so every bulk transfer is a
    # single contiguous 8.7KB run per partition (max DMA-engine efficiency);
    # image 3's last two stacks stay as single-stack transfers [H, (p w)]
    # for fine-grained tail chasing.
    dets_p = nc.dram_tensor(
        "dets_p", [7, H, 2 * FREE], bf16, kind="ExternalInput"
    )
    dets_s = nc.dram_tensor("dets_s", [2, H, FREE], bf16, kind="ExternalInput")
    heat_p = nc.dram_tensor("heat_p", [2, H, 2 * FREE], bf16, kind="ExternalInput")
    maskw = nc.dram_tensor("maskw", [H, BLOC * W], bf16, kind="ExternalInput")
    tagin = nc.dram_tensor("tagin", [128, 3 * TAG_COLS], f32, kind="ExternalInput")
    out_det = nc.dram_tensor(
        "out_det", [128, BLOC * 2 * 2 * WG], f32, kind="ExternalOutput"
    )
    out_tag = nc.dram_tensor("out_tag", [1, 1], f32, kind="ExternalOutput")

    with TileContext(nc) as tc:
        with (
            tc.tile_pool(name="const", bufs=1) as cpool,
            tc.tile_pool(name="heatp", bufs=2) as hpool,
            tc.tile_pool(name="detp", bufs=7) as dpool,
            tc.tile_pool(name="dets3", bufs=2) as spool,
            tc.tile_pool(name="diffp", bufs=3) as fpool,
            tc.tile_pool(name="diffs", bufs=2) as spool_f,
            tc.tile_pool(name="sqp", bufs=4) as qpool,
            tc.tile_pool(name="psum0", bufs=1, space="PSUM") as ppool0,
            tc.tile_pool(name="psum1", bufs=1, space="PSUM") as ppool1,
            tc.tile_pool(name="psum2", bufs=1, space="PSUM") as ppool2,
            tc.tile_pool(name="psum3", bufs=1, space="PSUM") as ppool3,
        ):
            # Small inputs ride the scalar HWDGE ring so they don't queue
            # behind the bulk det stream on the sync ring.
            mask_t = cpool.tile([128, BLOC * W], bf16)
            nc.scalar.dma_start(out=mask_t[:], in_=maskw[:])
            tag_t = cpool.tile([128, 3 * TAG_COLS], f32)
            nc.scalar.dma_start(out=tag_t[:], in_=tagin[:])

            # Dummy activation so the Square table set loads during the DMA
            # ramp instead of delaying the first real square.
            warm_t = cpool.tile([1, 8], f32)
            warm_o = cpool.tile([1, 8], f32)
            nc.gpsimd.memset(warm_t[:], 0.0)
            nc.scalar.activation(
                warm_o[:], warm_t[:], mybir.ActivationFunctionType.Square
            )

            # 11 transfers up-front on the sync ring in need-order, one tile
            # per transfer.  Measured facts behind this shape: (a) single
            # contiguous runs per partition are the efficient DMA unit (the
            # host interleave makes pairs one 8.7KB run); (b) multiple
            # outstanding DMAs into ONE tile serialize on that tile's
            # semaphore; (c) the dynamic-HWDGE path keeps ~12 transfers in
            # flight and stalls the issuing sequencer when full — slots free
            # progressively, so the last two (needed last) still arrive in
            # time.  Within a ring, descriptors drain FIFO.
            # Issue order: image 0/1 pairs, heat1, then image 3's two singles
            # MID-stream (their chains fill DVE's natural gaps, and the
            # stream tail — where the activity throttle may clip DMA — then
            # carries pairs that are mostly landed before it fires), then
            # image 2's pairs and image 3's s01 pair last.
            heat_tiles = {}
            det_tiles = {}
            pair_tiles = {}

            def emit_heat(b0):
                hp = hpool.tile([128, 2 * FREE], bf16, name="heat_t", tag="heat_t")
                nc.sync.dma_start(out=hp[:], in_=heat_p[b0 // 2])
                heat_tiles[b0] = hp[:, 0:FREE]
                heat_tiles[b0 + 1] = hp[:, FREE : 2 * FREE]

            def emit_pair(b, s0, pair_idx):
                dp = dpool.tile([128, 2 * FREE], bf16, name="det_t", tag="det_t")
                nc.sync.dma_start(out=dp[:], in_=dets_p[pair_idx])
                pair_tiles[(b, s0)] = dp[:]
                det_tiles[(b, s0)] = dp[:, 0:FREE]
                det_tiles[(b, s0 + 1)] = dp[:, FREE : 2 * FREE]

            emit_heat(0)
            emit_pair(0, 0, 0)
            emit_pair(0, 2, 1)
            emit_pair(1, 0, 2)
            emit_pair(1, 2, 3)
            emit_heat(2)
            for s in (2, 3):
                ds = spool.tile([128, FREE], bf16, name="det_s", tag="det_s")
                nc.sync.dma_start(out=ds[:], in_=dets_s[s - 2])
                det_tiles[(BLOC - 1, s)] = ds[:]
            emit_pair(2, 0, 4)
            emit_pair(2, 2, 5)
            emit_pair(3, 0, 6)

            acc_det = cpool.tile([128, BLOC * 2 * 2 * WG], f32)

            def tag_block():
                # Tag loss (tiny) on the POOL engine: gated only by the
                # slow scalar-ring tagin transfer, which Pool can wait on
                # without holding up the DVE det pipeline.  The out_tag DMA
                # issues from Pool (SWDGE) so its tag-semaphore wait can
                # never park the ACT or sync sequencers.
                ptg_t = tag_t[:, 0:TAG_COLS]
                gtv_t = tag_t[:, TAG_COLS : 2 * TAG_COLS]
                vis_t = tag_t[:, 2 * TAG_COLS : 3 * TAG_COLS]
                e_t = cpool.tile([128, TAG_COLS], f32)
                ev_t = cpool.tile([128, TAG_COLS], f32)
                scr_t = cpool.tile([128, TAG_COLS], f32)
                tag_acc = cpool.tile([1, 1], f32)
                nc.gpsimd.tensor_sub(e_t[:], ptg_t, gtv_t)
                nc.gpsimd.tensor_mul(ev_t[:], e_t[:], vis_t)
                nc.gpsimd.tensor_mul(scr_t[:], e_t[:], ev_t[:])
                nc.gpsimd.tensor_reduce(
                    tag_acc[:], scr_t[:],
                    axis=mybir.AxisListType.XYZWC, op=mybir.AluOpType.add,
                )
                nc.gpsimd.dma_start(out=out_tag[:], in_=tag_acc[:])

            # ---- heatmap (det) loss ----
            # Each image owns a 2-bank [128, 1024] f32 PSUM tile: the s01
            # round accumulates in region [0:272] (bank 0), s23 in [512:784]
            # (bank 1) — per-matmul accumulation never crosses a 2KB bank.
            # 16 matmuls per round, each covering 8 w-columns: lhsT = mask
            # 8-col slice, rhs = sq viewed [h, w8, s, p].  Group g targets
            # PE column quadrant g % 4 (tile_position) so consecutive
            # LDWEIGHTS+MATMUL pairs pipeline in different sub-arrays; the
            # 4 groups of a quadrant accumulate.  Useful outputs:
            # psum[32q+j, region + j-th 34-block].
            def emit_round(b, psum_t, sq_g, s0, s1):
                region = 0 if s0 < 2 else PSTRIDE
                psum_r = psum_t[:, region : region + RSQ].rearrange(
                    "m (j s p) -> m j s p", j=WG, s=2
                )
                srel0, srel1 = s0 % 2, (s1 - 1) % 2 + 1
                for g in range(NG):
                    q = g % 4
                    nc.tensor.matmul(
                        psum_r[32 * q : 32 * q + WG, :, srel0:srel1, :],
                        lhsT=mask_t[:, b * W + WG * g : b * W + WG * (g + 1)],
                        rhs=sq_g[:, WG * g : WG * (g + 1), s0:s1, :],
                        start=(g < 4),
                        stop=(g >= NG - 4),
                        tile_position=(0, 32 * q),
                    )

            ppools = [ppool0, ppool1, ppool2, ppool3]
            psum_ts = {}
            sq_ts = {}
            sq_gs = {}

            def img_state(b):
                sq_t = qpool.tile([128, S * FREE], bf16, name=f"sq{b}", tag="sq")
                psum_t = ppools[b].tile(
                    [128, 2 * PSTRIDE], f32, name=f"ps{b}", tag=f"ps{b}"
                )
                psum_ts[b] = psum_t
                sq_ts[b] = sq_t
                sq_gs[b] = sq_t[:].rearrange(
                    "q (s p w) -> q w s p", s=S, p=N_PARTS
                )

            def pair_unit(b, s0, sq_eng="act", round_=True):
                # one sub + one square per 2-stack slab (heat broadcast)
                heat_b = heat_tiles[b].unsqueeze(1).broadcast_to([128, 2, FREE])
                diff_t = fpool.tile(
                    [128, 2 * FREE], bf16, name="diffp_t", tag="diffp_t"
                )
                src3 = pair_tiles[(b, s0)].rearrange("p (s f) -> p s f", s=2)
                dst3 = diff_t[:].rearrange("p (s f) -> p s f", s=2)
                nc.vector.tensor_sub(dst3, src3, heat_b)
                sqdst = sq_ts[b][:, s0 * FREE : (s0 + 2) * FREE]
                if sq_eng == "act":
                    nc.scalar.activation(
                        sqdst, diff_t[:], mybir.ActivationFunctionType.Square
                    )
                else:
                    nc.vector.tensor_mul(sqdst, diff_t[:], diff_t[:])
                if round_:
                    emit_round(b, psum_ts[b], sq_gs[b], s0, s0 + 2)

            def single_sub(b, s):
                diff_t = spool_f.tile(
                    [128, FREE], bf16, name="diffs_t", tag="diffs_t"
                )
                nc.vector.tensor_sub(diff_t[:], det_tiles[(b, s)], heat_tiles[b])
                return diff_t

            def single_sq(b, s, diff_t):
                dst = sq_ts[b][:, s * FREE : (s + 1) * FREE]
                nc.vector.tensor_mul(dst, diff_t[:], diff_t[:])
                emit_round(b, psum_ts[b], sq_gs[b], s, s + 1)

            for b in range(BLOC):
                img_state(b)
            # Images 0-1: pairs, ACT squares, rounds inline.
            pair_unit(0, 0)
            pair_unit(0, 2)
            tag_block()
            pair_unit(1, 0)
            pair_unit(1, 2)
            # Image 3's s2/s3 arrive mid-stream: their whole chains fill
            # DVE's arrival gaps and retire PSUM bank-3's B region early.
            d32 = single_sub(3, 2)
            single_sq(3, 2, d32)
            d33 = single_sub(3, 3)
            single_sq(3, 3, d33)
            # Image 2: ACT squares; its s23 round is deferred so ACT's last
            # square cannot block image 3's s01 round in PE's static order.
            pair_unit(2, 0)
            pair_unit(2, 2, round_=False)
            # Image 3's s01 pair closes the stream: sub + square on DVE.
            pair_unit(3, 0, sq_eng="dve")
            emit_round(2, psum_ts[2], sq_gs[2], 2, 4)

            # Per-region p-axis reductions (DVE, ~0.45us each) emitted LAST:
            # each waits on its region's final matmul; the out DMA then
            # carries only [128, 128] f32.  Latest-dependency regions go
            # last so the static scheduler cannot park DVE on them.
            def reduce_region(b, region, off):
                src = psum_ts[b][:, region * PSTRIDE : region * PSTRIDE + RSQ]
                dst = acc_det[:, off : off + 2 * WG].rearrange(
                    "m (j s) -> m j s", j=WG
                )
                nc.vector.reduce_sum(
                    dst, src.rearrange("m (j s p) -> m j s p", j=WG, s=2),
                    axis=mybir.AxisListType.X,
                )

            order = [(0, 0), (0, 1), (1, 0), (1, 1), (3, 1), (2, 0), (3, 0), (2, 1)]
            for b, region in order:
                reduce_region(b, region, (b * 2 + region) * 2 * WG)

            # Output DMA issues from the sync sequencer, which is parked on
            # its wait by then and fires in parallel with the copy.
            nc.sync.dma_start(out=out_det[:], in_=acc_det[:])
    nc.compile()
    return nc


def _pack(vals):
    """vals: (BLOC, S, KP) float32 -> [128, TAG_COLS] with col = b*16+s*4+j,
    partition k holding element j*128+k of the zero-padded 512 vector."""
    padded = np.zeros((BLOC, S, KP_COLS * 128), np.float32)
    padded[..., :KP] = vals
    return (
        padded.reshape(BLOC, S, KP_COLS, 128)
        .transpose(3, 0, 1, 2)
        .reshape(128, TAG_COLS)
    )


def kernel(preds, masks, keypoints_idx, keypoints_vis, gt_tags, heatmaps):
    preds = np.asarray(preds, dtype=np.float32)
    masks = np.asarray(masks, dtype=np.float32)
    keypoints_idx = np.asarray(keypoints_idx)
    keypoints_vis = np.asarray(keypoints_vis, dtype=np.float32)
    gt_tags = np.asarray(gt_tags, dtype=np.float32)
    heatmaps = np.asarray(heatmaps, dtype=np.float32)

    if "nc" not in _cache:
        _cache["nc"] = _build()
    nc = _cache["nc"]

    # Host-side input staging: gather predicted tags at keypoint locations
    # (index-based staging; all loss arithmetic runs on device).
    tags = preds[:, :, N_PARTS:].reshape(B, S, N_PARTS * H * W)
    flat_idx = keypoints_idx.reshape(B, 1, KP).astype(np.int64)
    pt = np.take_along_axis(tags, np.broadcast_to(flat_idx, (B, S, KP)), axis=2)
    gt = gt_tags.reshape(B, KP)
    vi = keypoints_vis.reshape(B, KP)

    in_maps = []
    for c in range(NCORES):
        b0 = c * BLOC
        sl = slice(b0, b0 + BLOC)
        tag_in = np.concatenate(
            [
                _pack(pt[sl]),
                _pack(np.broadcast_to(gt[sl][:, None, :], (BLOC, S, KP))),
                _pack(np.broadcast_to(vi[sl][:, None, :], (BLOC, S, KP))),
            ],
            axis=1,
        )
        # [BLOC, S, 17, H, W] -> [BLOC, S, H, 17, W], then pack 2-stack
        # pairs as [H, (s p w)] and heat 2-image pairs as [H, (b p w)] so
        # each bulk DMA is one contiguous run per partition.
        dt_ = preds[sl, :, :N_PARTS].transpose(0, 1, 3, 2, 4)
        ht_ = heatmaps[sl].transpose(0, 2, 1, 3)
        dets_p = np.stack(
            [
                dt_[b, s0 : s0 + 2].transpose(1, 0, 2, 3).reshape(H, 2 * FREE)
                for b, s0 in (
                    (0, 0), (0, 2), (1, 0), (1, 2), (2, 0), (2, 2), (3, 0),
                )
            ]
        )
        dets_s = dt_[3, 2:4].reshape(2, H, FREE)
        heat_pk = np.stack(
            [
                ht_[b0 : b0 + 2].transpose(1, 0, 2, 3).reshape(H, 2 * FREE)
                for b0 in (0, 2)
            ]
        )
        in_maps.append(
            {
                "dets_p": np.ascontiguousarray(dets_p).astype(ml_dtypes.bfloat16),
                "dets_s": np.ascontiguousarray(dets_s).astype(ml_dtypes.bfloat16),
                "heat_p": np.ascontiguousarray(heat_pk).astype(ml_dtypes.bfloat16),
                # [BLOC, H, W] -> [H, BLOC*W]
                "maskw": np.ascontiguousarray(
                    masks[sl].transpose(1, 0, 2).reshape(H, BLOC * W)
                ).astype(ml_dtypes.bfloat16),
                "tagin": np.ascontiguousarray(tag_in),
            }
        )

    res = run_bass_kernel_spmd(nc, in_maps, list(range(NCORES)))
    _cache["last_results"] = res

    det_total = 0.0
    tag_total = 0.0
    for r in res.results:
        od = r["out_det"].astype(np.float64)
        # image b, region r2 (stack pair): row 32q+i, cols (j, s) with the
        # diag j==i holding the masked partial sums (p pre-reduced on DVE)
        for bi in range(BLOC):
            for r2 in range(2):
                for q in range(4):
                    for i in range(WG):
                        c0 = (bi * 2 + r2) * 2 * WG + i * 2
                        det_total += float(od[32 * q + i, c0 : c0 + 2].sum())
        tag_total += float(r["out_tag"].sum(dtype=np.float64))

    det_mean = det_total / (B * S * N_PARTS * H * W)
    tag_mean = tag_total / (B * S)
    return np.float32(TAG_W * tag_mean + HM_W * det_mean)


# revision 40
# speedup vs baseline: 1.0488x; 1.0144x over previous
"""Trainium2 Bass kernel for nn_LossSupervisedTags (tag + heatmap MSE loss).

Contract: kernel(**inputs) takes the FULL unsharded inputs (as produced by
setup_inputs) and returns the FULL scalar output.  Internally the batch dim
(B=32) is sharded 4-images-per-core across 8 NeuronCores; each core computes
its local tag / heatmap loss partial sums on device, and the host combines
the 8 partial sums into the final scalar mean.

Host staging: slices per-core shards, transposes dets/heat to [h, p, w] so
every DMA descriptor is a fat contiguous run, and gathers the 510 predicted
tags per image (index staging for the tag loss).

Per-core device pipeline (per image b, stacks s=0..3):
  DVE   : diff = dets[b,s] - heat[b]      (layout [h=128, (p,w)=2176], bf16 2x)
  ACT/DVE: sq  = diff^2 into sq_img[:, s*2176:...] as bf16
  PE    : 32 matmuls per image, each covering a group of 4 w-columns:
          lhsT = mask[:, 4g:4g+4] (bf16), rhs = sq viewed as [h, w4, s, p].
          Group g targets PE column quadrant g%4 (tile_position) so
          consecutive LDWEIGHTS+MATMUL pairs pipeline in different
          sub-arrays.  Each image accumulates into its OWN PSUM bank;
          right after image b's last matmul the bank is copied (DVE) into
          acc[:, b*272:...] so only image 3's chain sits in the tail.
          Image 3 runs one matmul round PER STACK (rhs 68 cols) so the
          final round starts as soon as the last stack's sq is ready.
  tag   : gathered pred tags packed [128,64]; (pt-gt)^2*vis summed on the
          POOL engine (gpsimd) so the slow scalar-ring tagin transfer never
          blocks the DVE pipeline.

DMA plan: 11 transfers on the sync HWDGE ring in need-order, one tile per
transfer (multiple outstanding DMAs into one tile serialize on its
semaphore).  mask/tagin and the tag output ride the scalar HWDGE ring;
descriptors within a ring drain FIFO.
"""

import sys
import types

import ml_dtypes
import numpy as np

import concourse.bacc as bacc
import concourse.mybir as mybir
from concourse.tile import TileContext
from concourse.bass_utils import run_bass_kernel_spmd

# If BASS_TRACE is set in the environment but this image lacks
# antenv.axon_hooks, run_bass_kernel_spmd would die on import; register a
# no-op hook module so tracing degrades gracefully instead.
try:
    import antenv.axon_hooks  # noqa: F401
except ImportError:
    try:
        import antenv

        _m = types.ModuleType("antenv.axon_hooks")
        _m.get_axon_ntff_profile_hook = lambda: None
        _m.set_axon_ntff_profile_hook = lambda h: None
        sys.modules["antenv.axon_hooks"] = _m
        antenv.axon_hooks = _m
    except ImportError:
        pass

# Problem constants (hardcoded per harness contract)
B, S, C, H, W = 32, 4, 34, 128, 128
N_PARTS, TAG_DIM, M = 17, 1, 30
TAG_W, HM_W = 0.001, 1.0
NCORES = 8
BLOC = B // NCORES            # 4 images per core
FREE = N_PARTS * W            # 2176 free elems per (b, s) tile
KP = M * N_PARTS              # 510 keypoints per image
KP_COLS = 4                   # ceil(510 / 128) columns per (b, s)
TAG_COLS = BLOC * S * KP_COLS  # 64
WG = 8                        # w-columns per matmul group
NG = W // WG                  # 16 matmul groups per round
RSQ = WG * 2 * N_PARTS        # 272 psum cols per (image, stack-pair) region
PSTRIDE = 512                 # psum region stride (bank-aligned, f32 elems)
OUTW = PSTRIDE + RSQ          # 784 cols copied out per image (incl. gap)

_cache = {}

# Elementwise plan.  Subs all ride DVE (only engine with 2-tensor bf16 2x)
# in pure arrival order — anything interleaved into DVE's queue delays later
# subs through the scheduler's quantized cross-engine waits.  ACT owns all
# six pair squares (kept ~75% duty, arrival-paced gaps dodge the DMA
# throttle); image 3's four squares land on DVE right after its subs (the
# stream is done by then, DVE is free, and the 1.3us/square latency beats
# ACT's 2.1 in the tail).


def _build():
    f32 = mybir.dt.float32
    bf16 = mybir.dt.bfloat16
    nc = bacc.Bacc(
        "TRN2", target_bir_lowering=False, debug=False, num_devices=NCORES
    )
    # Host pre-interleaves dets into 2-stack pairs [pair, H, (s p w)] and
    # heat into 2-image pairs [pair, H, (b p w)] so every bulk transfer is a
    # single contiguous 8.7KB run per partition (max DMA-engine efficiency);
    # image 3's last two stacks stay as single-stack transfers [H, (p w)]
    # for fine-grained tail chasing.
    dets_p = nc.dram_tensor(
        "dets_p", [7, H, 2 * FREE], bf16, kind="ExternalInput"
    )
    dets_s = nc.dram_tensor("dets_s", [2, H, FREE], bf16, kind="ExternalInput")
    heat_p = nc.dram_tensor("heat_p", [2, H, 2 * FREE], bf16, kind="ExternalInput")
    maskw = nc.dram_tensor("maskw", [H, BLOC * W], bf16, kind="ExternalInput")
    tagin = nc.dram_tensor("tagin", [128, 3 * TAG_COLS], f32, kind="ExternalInput")
    out_det = nc.dram_tensor(
        "out_det", [128, BLOC * 2 * 2 * WG], f32, kind="ExternalOutput"
    )
    out_tag = nc.dram_tensor("out_tag", [1, 1], f32, kind="ExternalOutput")

    with TileContext(nc) as tc:
        with (
            tc.tile_pool(name="const", bufs=1) as cpool,
            tc.tile_pool(name="heatp", bufs=2) as hpool,
            tc.tile_pool(name="detp", bufs=7) as dpool,
            tc.tile_pool(name="dets3", bufs=2) as spool,
            tc.tile_pool(name="diffp", bufs=3) as fpool,
            tc.tile_pool(name="diffs", bufs=2) as spool_f,
            tc.tile_pool(name="sqp", bufs=4) as qpool,
            tc.tile_pool(name="psum0", bufs=1, space="PSUM") as ppool0,
            tc.tile_pool(name="psum1", bufs=1, space="PSUM") as ppool1,
            tc.tile_pool(name="psum2", bufs=1, space="PSUM") as ppool2,
            tc.tile_pool(name="psum3", bufs=1, space="PSUM") as ppool3,
        ):
            # Small inputs ride the scalar HWDGE ring so they don't queue
            # behind the bulk det stream on the sync ring.
            mask_t = cpool.tile([128, BLOC * W], bf16)
            nc.scalar.dma_start(out=mask_t[:], in_=maskw[:])
            tag_t = cpool.tile([128, 3 * TAG_COLS], f32)
            nc.scalar.dma_start(out=tag_t[:], in_=tagin[:])

            # Dummy activation so the Square table set loads during the DMA
            # ramp instead of delaying the first real square.
            warm_t = cpool.tile([1, 8], f32)
            warm_o = cpool.tile([1, 8], f32)
            nc.gpsimd.memset(warm_t[:], 0.0)
            nc.scalar.activation(
                warm_o[:], warm_t[:], mybir.ActivationFunctionType.Square
            )

            # 11 transfers up-front on the sync ring in need-order, one tile
            # per transfer.  Measured facts behind this shape: (a) single
            # contiguous runs per partition are the efficient DMA unit (the
            # host interleave makes pairs one 8.7KB run); (b) multiple
            # outstanding DMAs into ONE tile serialize on that tile's
            # semaphore; (c) the dynamic-HWDGE path keeps ~12 transfers in
            # flight and stalls the issuing sequencer when full — slots free
            # progressively, so the last two (needed last) still arrive in
            # time.  Within a ring, descriptors drain FIFO.
            # Issue order: image 0/1 pairs, heat1, then image 3's two singles
            # MID-stream (their chains fill DVE's natural gaps, and the
            # stream tail — where the activity throttle may clip DMA — then
            # carries pairs that are mostly landed before it fires), then
            # image 2's pairs and image 3's s01 pair last.
            heat_tiles = {}
            det_tiles = {}
            pair_tiles = {}

            def emit_heat(b0):
                hp = hpool.tile([128, 2 * FREE], bf16, name="heat_t", tag="heat_t")
                nc.sync.dma_start(out=hp[:], in_=heat_p[b0 // 2])
                heat_tiles[b0] = hp[:, 0:FREE]
                heat_tiles[b0 + 1] = hp[:, FREE : 2 * FREE]

            def emit_pair(b, s0, pair_idx):
                dp = dpool.tile([128, 2 * FREE], bf16, name="det_t", tag="det_t")
                nc.sync.dma_start(out=dp[:], in_=dets_p[pair_idx])
                pair_tiles[(b, s0)] = dp[:]
                det_tiles[(b, s0)] = dp[:, 0:FREE]
                det_tiles[(b, s0 + 1)] = dp[:, FREE : 2 * FREE]

            emit_heat(0)
            emit_pair(0, 0, 0)
            emit_pair(0, 2, 1)
            emit_pair(1, 0, 2)
            emit_pair(1, 2, 3)
            emit_heat(2)
            for s in (2, 3):
                ds = spool.tile([128, FREE], bf16, name="det_s", tag="det_s")
                nc.sync.dma_start(out=ds[:], in_=dets_s[s - 2])
                det_tiles[(BLOC - 1, s)] = ds[:]
            emit_pair(2, 0, 4)
            emit_pair(2, 2, 5)
            emit_pair(3, 0, 6)

            acc_det = cpool.tile([128, BLOC * 2 * 2 * WG], f32)

            def tag_block():
                # Tag loss (tiny) on the POOL engine: gated only by the
                # slow scalar-ring tagin transfer, which Pool can wait on
                # without holding up the DVE det pipeline.  The out_tag DMA
                # issues from Pool (SWDGE) so its tag-semaphore wait can
                # never park the ACT or sync sequencers.
                ptg_t = tag_t[:, 0:TAG_COLS]
                gtv_t = tag_t[:, TAG_COLS : 2 * TAG_COLS]
                vis_t = tag_t[:, 2 * TAG_COLS : 3 * TAG_COLS]
                e_t = cpool.tile([128, TAG_COLS], f32)
                ev_t = cpool.tile([128, TAG_COLS], f32)
                scr_t = cpool.tile([128, TAG_COLS], f32)
                tag_acc = cpool.tile([1, 1], f32)
                nc.gpsimd.tensor_sub(e_t[:], ptg_t, gtv_t)
                nc.gpsimd.tensor_mul(ev_t[:], e_t[:], vis_t)
                nc.gpsimd.tensor_mul(scr_t[:], e_t[:], ev_t[:])
                nc.gpsimd.tensor_reduce(
                    tag_acc[:], scr_t[:],
                    axis=mybir.AxisListType.XYZWC, op=mybir.AluOpType.add,
                )
                nc.gpsimd.dma_start(out=out_tag[:], in_=tag_acc[:])

            # ---- heatmap (det) loss ----
            # Each image owns a 2-bank [128, 1024] f32 PSUM tile: the s01
            # round accumulates in region [0:272] (bank 0), s23 in [512:784]
            # (bank 1) — per-matmul accumulation never crosses a 2KB bank.
            # 16 matmuls per round, each covering 8 w-columns: lhsT = mask
            # 8-col slice, rhs = sq viewed [h, w8, s, p].  Group g targets
            # PE column quadrant g % 4 (tile_position) so consecutive
            # LDWEIGHTS+MATMUL pairs pipeline in different sub-arrays; the
            # 4 groups of a quadrant accumulate.  Useful outputs:
            # psum[32q+j, region + j-th 34-block].
            def emit_round(b, psum_t, sq_g, s0, s1):
                region = 0 if s0 < 2 else PSTRIDE
                psum_r = psum_t[:, region : region + RSQ].rearrange(
                    "m (j s p) -> m j s p", j=WG, s=2
                )
                srel0, srel1 = s0 % 2, (s1 - 1) % 2 + 1
                for g in range(NG):
                    q = g % 4
                    nc.tensor.matmul(
                        psum_r[32 * q : 32 * q + WG, :, srel0:srel1, :],
                        lhsT=mask_t[:, b * W + WG * g : b * W + WG * (g + 1)],
                        rhs=sq_g[:, WG * g : WG * (g + 1), s0:s1, :],
                        start=(g < 4),
                        stop=(g >= NG - 4),
                        tile_position=(0, 32 * q),
                    )

            ppools = [ppool0, ppool1, ppool2, ppool3]
            psum_ts = {}
            sq_ts = {}
            sq_gs = {}

            def img_state(b):
                sq_t = qpool.tile([128, S * FREE], bf16, name=f"sq{b}", tag="sq")
                psum_t = ppools[b].tile(
                    [128, 2 * PSTRIDE], f32, name=f"ps{b}", tag=f"ps{b}"
                )
                psum_ts[b] = psum_t
                sq_ts[b] = sq_t
                sq_gs[b] = sq_t[:].rearrange(
                    "q (s p w) -> q w s p", s=S, p=N_PARTS
                )

            def pair_unit(b, s0, sq_eng="act", round_=True, split_sq=False):
                # one sub + one square per 2-stack slab (heat broadcast)
                heat_b = heat_tiles[b].unsqueeze(1).broadcast_to([128, 2, FREE])
                diff_t = fpool.tile(
                    [128, 2 * FREE], bf16, name="diffp_t", tag="diffp_t"
                )
                src3 = pair_tiles[(b, s0)].rearrange("p (s f) -> p s f", s=2)
                dst3 = diff_t[:].rearrange("p (s f) -> p s f", s=2)
                nc.vector.tensor_sub(dst3, src3, heat_b)
                if split_sq:
                    for ds in (0, 1):
                        s = s0 + ds
                        nc.vector.tensor_mul(
                            sq_ts[b][:, s * FREE : (s + 1) * FREE],
                            diff_t[:, ds * FREE : (ds + 1) * FREE],
                            diff_t[:, ds * FREE : (ds + 1) * FREE],
                        )
                        emit_round(b, psum_ts[b], sq_gs[b], s, s + 1)
                    return
                sqdst = sq_ts[b][:, s0 * FREE : (s0 + 2) * FREE]
                if sq_eng == "act":
                    nc.scalar.activation(
                        sqdst, diff_t[:], mybir.ActivationFunctionType.Square
                    )
                else:
                    nc.vector.tensor_mul(sqdst, diff_t[:], diff_t[:])
                if round_:
                    emit_round(b, psum_ts[b], sq_gs[b], s0, s0 + 2)

            def single_sub(b, s):
                diff_t = spool_f.tile(
                    [128, FREE], bf16, name="diffs_t", tag="diffs_t"
                )
                nc.vector.tensor_sub(diff_t[:], det_tiles[(b, s)], heat_tiles[b])
                return diff_t

            def single_sq(b, s, diff_t):
                dst = sq_ts[b][:, s * FREE : (s + 1) * FREE]
                nc.vector.tensor_mul(dst, diff_t[:], diff_t[:])
                emit_round(b, psum_ts[b], sq_gs[b], s, s + 1)

            for b in range(BLOC):
                img_state(b)
            # Images 0-1: pairs, ACT squares, rounds inline.
            pair_unit(0, 0)
            pair_unit(0, 2)
            tag_block()
            pair_unit(1, 0)
            pair_unit(1, 2)
            # Image 3's s2/s3 arrive mid-stream: their whole chains fill
            # DVE's arrival gaps and retire PSUM bank-3's B region early.
            d32 = single_sub(3, 2)
            single_sq(3, 2, d32)
            d33 = single_sub(3, 3)
            single_sq(3, 3, d33)
            # Image 2: ACT squares.  The s23 slab squares as two singles
            # (2.1us grains instead of one 3.9) with per-stack rounds, so
            # PE's last image-2 round starts and ends earlier; rounds are
            # deferred past image 3's so they cannot block it in PE's
            # static order.
            pair_unit(2, 0)
            d22 = pair_tiles[(2, 2)]
            diff22 = fpool.tile([128, 2 * FREE], bf16, name="diffp_t", tag="diffp_t")
            heat_b2 = heat_tiles[2].unsqueeze(1).broadcast_to([128, 2, FREE])
            nc.vector.tensor_sub(
                diff22[:].rearrange("p (s f) -> p s f", s=2),
                d22.rearrange("p (s f) -> p s f", s=2),
                heat_b2,
            )
            for s in (2, 3):
                nc.scalar.activation(
                    sq_ts[2][:, s * FREE : (s + 1) * FREE],
                    diff22[:, (s - 2) * FREE : (s - 1) * FREE],
                    mybir.ActivationFunctionType.Square,
                )
            # Image 3's s01 pair closes the stream: one sub, two DVE single
            # squares + per-stack rounds (shortest closing PE grains).
            pair_unit(3, 0, sq_eng="dve", round_=False, split_sq=True)
            emit_round(2, psum_ts[2], sq_gs[2], 2, 3)
            emit_round(2, psum_ts[2], sq_gs[2], 3, 4)

            # Per-region p-axis reductions (DVE, ~0.45us each) emitted LAST:
            # each waits on its region's final matmul; the out DMA then
            # carries only [128, 128] f32.  Latest-dependency regions go
            # last so the static scheduler cannot park DVE on them.
            def reduce_region(b, region, off):
                src = psum_ts[b][:, region * PSTRIDE : region * PSTRIDE + RSQ]
                dst = acc_det[:, off : off + 2 * WG].rearrange(
                    "m (j s) -> m j s", j=WG
                )
                nc.vector.reduce_sum(
                    dst, src.rearrange("m (j s p) -> m j s p", j=WG, s=2),
                    axis=mybir.AxisListType.X,
                )

            order = [(0, 0), (0, 1), (1, 0), (1, 1), (3, 1), (2, 0), (3, 0), (2, 1)]
            for b, region in order:
                reduce_region(b, region, (b * 2 + region) * 2 * WG)

            # Output DMA issues from the sync sequencer, which is parked on
            # its wait by then and fires in parallel with the copy.
            nc.sync.dma_start(out=out_det[:], in_=acc_det[:])
    nc.compile()
    return nc


def _pack(vals):
    """vals: (BLOC, S, KP) float32 -> [128, TAG_COLS] with col = b*16+s*4+j,
    partition k holding element j*128+k of the zero-padded 512 vector."""
    padded = np.zeros((BLOC, S, KP_COLS * 128), np.float32)
    padded[..., :KP] = vals
    return (
        padded.reshape(BLOC, S, KP_COLS, 128)
        .transpose(3, 0, 1, 2)
        .reshape(128, TAG_COLS)
    )


def kernel(preds, masks, keypoints_idx, keypoints_vis, gt_tags, heatmaps):
    preds = np.asarray(preds, dtype=np.float32)
    masks = np.asarray(masks, dtype=np.float32)
    keypoints_idx = np.asarray(keypoints_idx)
    keypoints_vis = np.asarray(keypoints_vis, dtype=np.float32)
    gt_tags = np.asarray(gt_tags, dtype=np.float32)
    heatmaps = np.asarray(heatmaps, dtype=np.float32)

    if "nc" not in _cache:
        _cache["nc"] = _build()
    nc = _cache["nc"]

    # Host-side input staging: gather predicted tags at keypoint locations
    # (index-based staging; all loss arithmetic runs on device).
    tags = preds[:, :, N_PARTS:].reshape(B, S, N_PARTS * H * W)
    flat_idx = keypoints_idx.reshape(B, 1, KP).astype(np.int64)
    pt = np.take_along_axis(tags, np.broadcast_to(flat_idx, (B, S, KP)), axis=2)
    gt = gt_tags.reshape(B, KP)
    vi = keypoints_vis.reshape(B, KP)

    in_maps = []
    for c in range(NCORES):
        b0 = c * BLOC
        sl = slice(b0, b0 + BLOC)
        tag_in = np.concatenate(
            [
                _pack(pt[sl]),
                _pack(np.broadcast_to(gt[sl][:, None, :], (BLOC, S, KP))),
                _pack(np.broadcast_to(vi[sl][:, None, :], (BLOC, S, KP))),
            ],
            axis=1,
        )
        # [BLOC, S, 17, H, W] -> [BLOC, S, H, 17, W], then pack 2-stack
        # pairs as [H, (s p w)] and heat 2-image pairs as [H, (b p w)] so
        # each bulk DMA is one contiguous run per partition.
        dt_ = preds[sl, :, :N_PARTS].transpose(0, 1, 3, 2, 4)
        ht_ = heatmaps[sl].transpose(0, 2, 1, 3)
        dets_p = np.stack(
            [
                dt_[b, s0 : s0 + 2].transpose(1, 0, 2, 3).reshape(H, 2 * FREE)
                for b, s0 in (
                    (0, 0), (0, 2), (1, 0), (1, 2), (2, 0), (2, 2), (3, 0),
                )
            ]
        )
        dets_s = dt_[3, 2:4].reshape(2, H, FREE)
        heat_pk = np.stack(
            [
                ht_[b0 : b0 + 2].transpose(1, 0, 2, 3).reshape(H, 2 * FREE)
                for b0 in (0, 2)
            ]
        )
        in_maps.append(
            {
                "dets_p": np.ascontiguousarray(dets_p).astype(ml_dtypes.bfloat16),
                "dets_s": np.ascontiguousarray(dets_s).astype(ml_dtypes.bfloat16),
                "heat_p": np.ascontiguousarray(heat_pk).astype(ml_dtypes.bfloat16),
                # [BLOC, H, W] -> [H, BLOC*W]
                "maskw": np.ascontiguousarray(
                    masks[sl].transpose(1, 0, 2).reshape(H, BLOC * W)
                ).astype(ml_dtypes.bfloat16),
                "tagin": np.ascontiguousarray(tag_in),
            }
        )

    res = run_bass_kernel_spmd(nc, in_maps, list(range(NCORES)))
    _cache["last_results"] = res

    det_total = 0.0
    tag_total = 0.0
    for r in res.results:
        od = r["out_det"].astype(np.float64)
        # image b, region r2 (stack pair): row 32q+i, cols (j, s) with the
        # diag j==i holding the masked partial sums (p pre-reduced on DVE)
        for bi in range(BLOC):
            for r2 in range(2):
                for q in range(4):
                    for i in range(WG):
                        c0 = (bi * 2 + r2) * 2 * WG + i * 2
                        det_total += float(od[32 * q + i, c0 : c0 + 2].sum())
        tag_total += float(r["out_tag"].sum(dtype=np.float64))

    det_mean = det_total / (B * S * N_PARTS * H * W)
    tag_mean = tag_total / (B * S)
    return np.float32(TAG_W * tag_mean + HM_W * det_mean)


# revision 42
# speedup vs baseline: 1.0498x; 1.0009x over previous
"""Trainium2 Bass kernel for nn_LossSupervisedTags (tag + heatmap MSE loss).

Contract: kernel(**inputs) takes the FULL unsharded inputs (as produced by
setup_inputs) and returns the FULL scalar output.  Internally the batch dim
(B=32) is sharded 4-images-per-core across 8 NeuronCores; each core computes
its local tag / heatmap loss partial sums on device, and the host combines
the 8 partial sums into the final scalar mean.

Host staging: slices per-core shards, transposes dets/heat to [h, p, w] so
every DMA descriptor is a fat contiguous run, and gathers the 510 predicted
tags per image (index staging for the tag loss).

Per-core device pipeline:
  DVE   : diff = dets - heat on 2-stack PAIR slabs [h=128, (s,p,w)=4352]
          (heat broadcast via 0-stride AP; bf16 2x) in pure arrival order —
          DVE's queue is kept free of anything that could park it on a
          slower engine's semaphore (measured ~2-4us bubbles otherwise).
  ACT   : squares (table Square) for images 0-2 — ~75% duty with
          arrival-paced gaps; ~6us of *sustained simultaneous* DVE+ACT
          density trips an activity throttle that clamps the DMA fabric
          ~4x (and deeper states also downclock engines ~20%).
  PE    : per (image, stack-pair) matmul rounds of 16 matmuls, each
          covering 8 w-columns: lhsT = mask 8-col slice, rhs = sq viewed
          [h, w8, s, p]; group g targets PE column quadrant g%4
          (tile_position) so LDWEIGHTS+MATMUL pairs pipeline in different
          sub-arrays.  Each image owns a 2-bank [128,1024] f32 PSUM tile
          (region s01 at [0:272], s23 at [512:784]) — matmul accumulation
          never crosses a 2KB bank.  PE runs at its low p-state
          (~2.8ns/col), so rounds are split fine-grained and the two
          closing rounds are single-stack (8 matmuls' worth of cols).
  DVE   : per-region reduce over parts (axis X) into a [128,128] f32
          accumulator — the out DMA then moves only 65KB.
  tag   : gathered pred tags packed [128,64]; (pt-gt)^2*vis reduced on the
          POOL engine (gpsimd) with its out_tag DMA on SWDGE, so the slow
          scalar-ring tagin transfer never blocks DVE or ACT.

DMA plan (sync HWDGE ring, one tile per transfer): heat0, image-0/1 pairs,
heat1, image-3's two SINGLE stacks mid-stream (their chains fill DVE's
arrival gaps and the throttle-exposed stream tail then carries pairs that
are mostly landed), image-2 pairs, image-3's s01 pair last (its chain is
the shortest tail).  mask/tagin ride the scalar HWDGE ring.

Endgame: image 2's s23 squares run as two ACT singles with per-stack
rounds, image 3's s01 squares as two DVE singles with per-stack rounds;
deferred rounds/reduces are emitted last, ordered by dependency readiness
(the static scheduler re-orders with an optimistic PE model — emission
position is the only reliable control).
"""

import sys
import types

import ml_dtypes
import numpy as np

import concourse.bacc as bacc
import concourse.mybir as mybir
from concourse.tile import TileContext
from concourse.bass_utils import run_bass_kernel_spmd

# If BASS_TRACE is set in the environment but this image lacks
# antenv.axon_hooks, run_bass_kernel_spmd would die on import; register a
# no-op hook module so tracing degrades gracefully instead.
try:
    import antenv.axon_hooks  # noqa: F401
except ImportError:
    try:
        import antenv

        _m = types.ModuleType("antenv.axon_hooks")
        _m.get_axon_ntff_profile_hook = lambda: None
        _m.set_axon_ntff_profile_hook = lambda h: None
        sys.modules["antenv.axon_hooks"] = _m
        antenv.axon_hooks = _m
    except ImportError:
        pass

# Problem constants (hardcoded per harness contract)
B, S, C, H, W = 32, 4, 34, 128, 128
N_PARTS, TAG_DIM, M = 17, 1, 30
TAG_W, HM_W = 0.001, 1.0
NCORES = 8
BLOC = B // NCORES            # 4 images per core
FREE = N_PARTS * W            # 2176 free elems per (b, s) tile
KP = M * N_PARTS              # 510 keypoints per image
KP_COLS = 4                   # ceil(510 / 128) columns per (b, s)
TAG_COLS = BLOC * S * KP_COLS  # 64
WG = 8                        # w-columns per matmul group
NG = W // WG                  # 16 matmul groups per round
RSQ = WG * 2 * N_PARTS        # 272 psum cols per (image, stack-pair) region
PSTRIDE = 512                 # psum region stride (bank-aligned, f32 elems)

_cache = {}

# Elementwise plan.  Subs all ride DVE (only engine with 2-tensor bf16 2x)
# in pure arrival order — anything interleaved into DVE's queue delays later
# subs through the scheduler's quantized cross-engine waits.  ACT owns all
# six pair squares (kept ~75% duty, arrival-paced gaps dodge the DMA
# throttle); image 3's four squares land on DVE right after its subs (the
# stream is done by then, DVE is free, and the 1.3us/square latency beats
# ACT's 2.1 in the tail).


def _build():
    f32 = mybir.dt.float32
    bf16 = mybir.dt.bfloat16
    nc = bacc.Bacc(
        "TRN2", target_bir_lowering=False, debug=False, num_devices=NCORES
    )
    # Host pre-interleaves dets into 2-stack pairs [pair, H, (s p w)] and
    # heat into 2-image pairs [pair, H, (b p w)] so every bulk transfer is a
    # single contiguous 8.7KB run per partition (max DMA-engine efficiency);
    # image 3's last two stacks stay as single-stack transfers [H, (p w)]
    # for fine-grained tail chasing.
    dets_p = nc.dram_tensor(
        "dets_p", [7, H, 2 * FREE], bf16, kind="ExternalInput"
    )
    dets_s = nc.dram_tensor("dets_s", [2, H, FREE], bf16, kind="ExternalInput")
    heat_p = nc.dram_tensor("heat_p", [2, H, 2 * FREE], bf16, kind="ExternalInput")
    maskw = nc.dram_tensor("maskw", [H, BLOC * W], bf16, kind="ExternalInput")
    tagin = nc.dram_tensor("tagin", [128, 3 * TAG_COLS], f32, kind="ExternalInput")
    out_det = nc.dram_tensor(
        "out_det", [128, BLOC * 2 * 2 * WG], f32, kind="ExternalOutput"
    )
    out_tag = nc.dram_tensor("out_tag", [1, 1], f32, kind="ExternalOutput")

    with TileContext(nc) as tc:
        with (
            tc.tile_pool(name="const", bufs=1) as cpool,
            tc.tile_pool(name="heatp", bufs=2) as hpool,
            tc.tile_pool(name="detp", bufs=7) as dpool,
            tc.tile_pool(name="dets3", bufs=2) as spool,
            tc.tile_pool(name="diffp", bufs=3) as fpool,
            tc.tile_pool(name="diffs", bufs=2) as spool_f,
            tc.tile_pool(name="sqp", bufs=4) as qpool,
            tc.tile_pool(name="psum0", bufs=1, space="PSUM") as ppool0,
            tc.tile_pool(name="psum1", bufs=1, space="PSUM") as ppool1,
            tc.tile_pool(name="psum2", bufs=1, space="PSUM") as ppool2,
            tc.tile_pool(name="psum3", bufs=1, space="PSUM") as ppool3,
        ):
            # Small inputs ride the scalar HWDGE ring so they don't queue
            # behind the bulk det stream on the sync ring.
            mask_t = cpool.tile([128, BLOC * W], bf16)
            nc.scalar.dma_start(out=mask_t[:], in_=maskw[:])
            tag_t = cpool.tile([128, 3 * TAG_COLS], f32)
            nc.scalar.dma_start(out=tag_t[:], in_=tagin[:])

            # Dummy activation so the Square table set loads during the DMA
            # ramp instead of delaying the first real square.
            warm_t = cpool.tile([1, 8], f32)
            warm_o = cpool.tile([1, 8], f32)
            nc.gpsimd.memset(warm_t[:], 0.0)
            nc.scalar.activation(
                warm_o[:], warm_t[:], mybir.ActivationFunctionType.Square
            )

            # 11 transfers up-front on the sync ring in need-order, one tile
            # per transfer.  Measured facts behind this shape: (a) single
            # contiguous runs per partition are the efficient DMA unit (the
            # host interleave makes pairs one 8.7KB run); (b) multiple
            # outstanding DMAs into ONE tile serialize on that tile's
            # semaphore; (c) the dynamic-HWDGE path keeps ~12 transfers in
            # flight and stalls the issuing sequencer when full — slots free
            # progressively, so the last two (needed last) still arrive in
            # time.  Within a ring, descriptors drain FIFO.
            # Issue order: image 0/1 pairs, heat1, then image 3's two singles
            # MID-stream (their chains fill DVE's natural gaps, and the
            # stream tail — where the activity throttle may clip DMA — then
            # carries pairs that are mostly landed before it fires), then
            # image 2's pairs and image 3's s01 pair last.
            heat_tiles = {}
            det_tiles = {}
            pair_tiles = {}

            def emit_heat(b0):
                hp = hpool.tile([128, 2 * FREE], bf16, name="heat_t", tag="heat_t")
                nc.sync.dma_start(out=hp[:], in_=heat_p[b0 // 2])
                heat_tiles[b0] = hp[:, 0:FREE]
                heat_tiles[b0 + 1] = hp[:, FREE : 2 * FREE]

            def emit_pair(b, s0, pair_idx):
                dp = dpool.tile([128, 2 * FREE], bf16, name="det_t", tag="det_t")
                nc.sync.dma_start(out=dp[:], in_=dets_p[pair_idx])
                pair_tiles[(b, s0)] = dp[:]
                det_tiles[(b, s0)] = dp[:, 0:FREE]
                det_tiles[(b, s0 + 1)] = dp[:, FREE : 2 * FREE]

            emit_heat(0)
            emit_pair(0, 0, 0)
            emit_pair(0, 2, 1)
            emit_pair(1, 0, 2)
            emit_pair(1, 2, 3)
            emit_heat(2)
            for s in (2, 3):
                ds = spool.tile([128, FREE], bf16, name="det_s", tag="det_s")
                nc.sync.dma_start(out=ds[:], in_=dets_s[s - 2])
                det_tiles[(BLOC - 1, s)] = ds[:]
            emit_pair(2, 0, 4)
            emit_pair(2, 2, 5)
            emit_pair(3, 0, 6)

            acc_det = cpool.tile([128, BLOC * 2 * 2 * WG], f32)

            def tag_block():
                # Tag loss (tiny) on the POOL engine: gated only by the
                # slow scalar-ring tagin transfer, which Pool can wait on
                # without holding up the DVE det pipeline.  The out_tag DMA
                # issues from Pool (SWDGE) so its tag-semaphore wait can
                # never park the ACT or sync sequencers.
                ptg_t = tag_t[:, 0:TAG_COLS]
                gtv_t = tag_t[:, TAG_COLS : 2 * TAG_COLS]
                vis_t = tag_t[:, 2 * TAG_COLS : 3 * TAG_COLS]
                e_t = cpool.tile([128, TAG_COLS], f32)
                ev_t = cpool.tile([128, TAG_COLS], f32)
                scr_t = cpool.tile([128, TAG_COLS], f32)
                tag_acc = cpool.tile([1, 1], f32)
                nc.gpsimd.tensor_sub(e_t[:], ptg_t, gtv_t)
                nc.gpsimd.tensor_mul(ev_t[:], e_t[:], vis_t)
                nc.gpsimd.tensor_mul(scr_t[:], e_t[:], ev_t[:])
                nc.gpsimd.tensor_reduce(
                    tag_acc[:], scr_t[:],
                    axis=mybir.AxisListType.XYZWC, op=mybir.AluOpType.add,
                )
                nc.gpsimd.dma_start(out=out_tag[:], in_=tag_acc[:])

            # ---- heatmap (det) loss ----
            # Each image owns a 2-bank [128, 1024] f32 PSUM tile: the s01
            # round accumulates in region [0:272] (bank 0), s23 in [512:784]
            # (bank 1) — per-matmul accumulation never crosses a 2KB bank.
            # 16 matmuls per round, each covering 8 w-columns: lhsT = mask
            # 8-col slice, rhs = sq viewed [h, w8, s, p].  Group g targets
            # PE column quadrant g % 4 (tile_position) so consecutive
            # LDWEIGHTS+MATMUL pairs pipeline in different sub-arrays; the
            # 4 groups of a quadrant accumulate.  Useful outputs:
            # psum[32q+j, region + j-th 34-block].
            def emit_round(b, psum_t, sq_g, s0, s1):
                region = 0 if s0 < 2 else PSTRIDE
                psum_r = psum_t[:, region : region + RSQ].rearrange(
                    "m (j s p) -> m j s p", j=WG, s=2
                )
                srel0, srel1 = s0 % 2, (s1 - 1) % 2 + 1
                for g in range(NG):
                    q = g % 4
                    nc.tensor.matmul(
                        psum_r[32 * q : 32 * q + WG, :, srel0:srel1, :],
                        lhsT=mask_t[:, b * W + WG * g : b * W + WG * (g + 1)],
                        rhs=sq_g[:, WG * g : WG * (g + 1), s0:s1, :],
                        start=(g < 4),
                        stop=(g >= NG - 4),
                        tile_position=(0, 32 * q),
                    )

            ppools = [ppool0, ppool1, ppool2, ppool3]
            psum_ts = {}
            sq_ts = {}
            sq_gs = {}

            def img_state(b):
                sq_t = qpool.tile([128, S * FREE], bf16, name=f"sq{b}", tag="sq")
                psum_t = ppools[b].tile(
                    [128, 2 * PSTRIDE], f32, name=f"ps{b}", tag=f"ps{b}"
                )
                psum_ts[b] = psum_t
                sq_ts[b] = sq_t
                sq_gs[b] = sq_t[:].rearrange(
                    "q (s p w) -> q w s p", s=S, p=N_PARTS
                )

            def pair_unit(b, s0, sq_eng="act", round_=True, split_sq=False):
                # one sub + one square per 2-stack slab (heat broadcast)
                heat_b = heat_tiles[b].unsqueeze(1).broadcast_to([128, 2, FREE])
                diff_t = fpool.tile(
                    [128, 2 * FREE], bf16, name="diffp_t", tag="diffp_t"
                )
                src3 = pair_tiles[(b, s0)].rearrange("p (s f) -> p s f", s=2)
                dst3 = diff_t[:].rearrange("p (s f) -> p s f", s=2)
                nc.vector.tensor_sub(dst3, src3, heat_b)
                if split_sq:
                    for ds in (0, 1):
                        s = s0 + ds
                        nc.vector.tensor_mul(
                            sq_ts[b][:, s * FREE : (s + 1) * FREE],
                            diff_t[:, ds * FREE : (ds + 1) * FREE],
                            diff_t[:, ds * FREE : (ds + 1) * FREE],
                        )
                        emit_round(b, psum_ts[b], sq_gs[b], s, s + 1)
                    return
                sqdst = sq_ts[b][:, s0 * FREE : (s0 + 2) * FREE]
                if sq_eng == "act":
                    nc.scalar.activation(
                        sqdst, diff_t[:], mybir.ActivationFunctionType.Square
                    )
                else:
                    nc.vector.tensor_mul(sqdst, diff_t[:], diff_t[:])
                if round_:
                    emit_round(b, psum_ts[b], sq_gs[b], s0, s0 + 2)

            def single_sub(b, s):
                diff_t = spool_f.tile(
                    [128, FREE], bf16, name="diffs_t", tag="diffs_t"
                )
                nc.vector.tensor_sub(diff_t[:], det_tiles[(b, s)], heat_tiles[b])
                return diff_t

            def single_sq(b, s, diff_t):
                dst = sq_ts[b][:, s * FREE : (s + 1) * FREE]
                nc.vector.tensor_mul(dst, diff_t[:], diff_t[:])
                emit_round(b, psum_ts[b], sq_gs[b], s, s + 1)

            for b in range(BLOC):
                img_state(b)
            # Images 0-1: pairs, ACT squares, rounds inline.
            pair_unit(0, 0)
            pair_unit(0, 2)
            tag_block()
            pair_unit(1, 0)
            pair_unit(1, 2)
            # Image 3's s2/s3 arrive mid-stream: their whole chains fill
            # DVE's arrival gaps and retire PSUM bank-3's B region early.
            d32 = single_sub(3, 2)
            single_sq(3, 2, d32)
            d33 = single_sub(3, 3)
            single_sq(3, 3, d33)
            # Image 2: ACT squares.  The s23 slab squares as two singles
            # (2.1us grains instead of one 3.9) with per-stack rounds, so
            # PE's last image-2 round starts and ends earlier; rounds are
            # deferred past image 3's so they cannot block it in PE's
            # static order.
            pair_unit(2, 0)
            d22 = pair_tiles[(2, 2)]
            diff22 = fpool.tile([128, 2 * FREE], bf16, name="diffp_t", tag="diffp_t")
            heat_b2 = heat_tiles[2].unsqueeze(1).broadcast_to([128, 2, FREE])
            nc.vector.tensor_sub(
                diff22[:].rearrange("p (s f) -> p s f", s=2),
                d22.rearrange("p (s f) -> p s f", s=2),
                heat_b2,
            )
            for s in (2, 3):
                nc.scalar.activation(
                    sq_ts[2][:, s * FREE : (s + 1) * FREE],
                    diff22[:, (s - 2) * FREE : (s - 1) * FREE],
                    mybir.ActivationFunctionType.Square,
                )
            # Image 3's s01 pair closes the stream: one sub, two DVE single
            # squares + per-stack rounds (shortest closing PE grains).
            pair_unit(3, 0, sq_eng="dve", round_=False, split_sq=True)
            emit_round(2, psum_ts[2], sq_gs[2], 2, 3)
            emit_round(2, psum_ts[2], sq_gs[2], 3, 4)

            # Per-region p-axis reductions (DVE, ~0.45us each) emitted LAST:
            # each waits on its region's final matmul; the out DMA then
            # carries only [128, 128] f32.  Latest-dependency regions go
            # last so the static scheduler cannot park DVE on them.
            def reduce_region(b, region, off):
                src = psum_ts[b][:, region * PSTRIDE : region * PSTRIDE + RSQ]
                dst = acc_det[:, off : off + 2 * WG].rearrange(
                    "m (j s) -> m j s", j=WG
                )
                nc.vector.reduce_sum(
                    dst, src.rearrange("m (j s p) -> m j s p", j=WG, s=2),
                    axis=mybir.AxisListType.X,
                )

            order = [(0, 0), (0, 1), (1, 0), (1, 1), (3, 1), (2, 0), (3, 0), (2, 1)]
            for b, region in order:
                reduce_region(b, region, (b * 2 + region) * 2 * WG)

            # Output DMA issues from the sync sequencer, which is parked on
            # its wait by then and fires in parallel with the copy.
            nc.sync.dma_start(out=out_det[:], in_=acc_det[:])
    nc.compile()
    return nc


def _pack(vals):
    """vals: (BLOC, S, KP) float32 -> [128, TAG_COLS] with col = b*16+s*4+j,
    partition k holding element j*128+k of the zero-padded 512 vector."""
    padded = np.zeros((BLOC, S, KP_COLS * 128), np.float32)
    padded[..., :KP] = vals
    return (
        padded.reshape(BLOC, S, KP_COLS, 128)
        .transpose(3, 0, 1, 2)
        .reshape(128, TAG_COLS)
    )


def kernel(preds, masks, keypoints_idx, keypoints_vis, gt_tags, heatmaps):
    preds = np.asarray(preds, dtype=np.float32)
    masks = np.asarray(masks, dtype=np.float32)
    keypoints_idx = np.asarray(keypoints_idx)
    keypoints_vis = np.asarray(keypoints_vis, dtype=np.float32)
    gt_tags = np.asarray(gt_tags, dtype=np.float32)
    heatmaps = np.asarray(heatmaps, dtype=np.float32)

    if "nc" not in _cache:
        _cache["nc"] = _build()
    nc = _cache["nc"]

    # Host-side input staging: gather predicted tags at keypoint locations
    # (index-based staging; all loss arithmetic runs on device).
    tags = preds[:, :, N_PARTS:].reshape(B, S, N_PARTS * H * W)
    flat_idx = keypoints_idx.reshape(B, 1, KP).astype(np.int64)
    pt = np.take_along_axis(tags, np.broadcast_to(flat_idx, (B, S, KP)), axis=2)
    gt = gt_tags.reshape(B, KP)
    vi = keypoints_vis.reshape(B, KP)

    in_maps = []
    for c in range(NCORES):
        b0 = c * BLOC
        sl = slice(b0, b0 + BLOC)
        tag_in = np.concatenate(
            [
                _pack(pt[sl]),
                _pack(np.broadcast_to(gt[sl][:, None, :], (BLOC, S, KP))),
                _pack(np.broadcast_to(vi[sl][:, None, :], (BLOC, S, KP))),
            ],
            axis=1,
        )
        # [BLOC, S, 17, H, W] -> [BLOC, S, H, 17, W], then pack 2-stack
        # pairs as [H, (s p w)] and heat 2-image pairs as [H, (b p w)] so
        # each bulk DMA is one contiguous run per partition.
        dt_ = preds[sl, :, :N_PARTS].transpose(0, 1, 3, 2, 4)
        ht_ = heatmaps[sl].transpose(0, 2, 1, 3)
        dets_p = np.stack(
            [
                dt_[b, s0 : s0 + 2].transpose(1, 0, 2, 3).reshape(H, 2 * FREE)
                for b, s0 in (
                    (0, 0), (0, 2), (1, 0), (1, 2), (2, 0), (2, 2), (3, 0),
                )
            ]
        )
        dets_s = dt_[3, 2:4].reshape(2, H, FREE)
        heat_pk = np.stack(
            [
                ht_[b0 : b0 + 2].transpose(1, 0, 2, 3).reshape(H, 2 * FREE)
                for b0 in (0, 2)
            ]
        )
        in_maps.append(
            {
                "dets_p": np.ascontiguousarray(dets_p).astype(ml_dtypes.bfloat16),
                "dets_s": np.ascontiguousarray(dets_s).astype(ml_dtypes.bfloat16),
                "heat_p": np.ascontiguousarray(heat_pk).astype(ml_dtypes.bfloat16),
                # [BLOC, H, W] -> [H, BLOC*W]
                "maskw": np.ascontiguousarray(
                    masks[sl].transpose(1, 0, 2).reshape(H, BLOC * W)
                ).astype(ml_dtypes.bfloat16),
                "tagin": np.ascontiguousarray(tag_in),
            }
        )

    res = run_bass_kernel_spmd(nc, in_maps, list(range(NCORES)))
    _cache["last_results"] = res

    det_total = 0.0
    tag_total = 0.0
    for r in res.results:
        od = r["out_det"].astype(np.float64)
        # image b, region r2 (stack pair): row 32q+i, cols (j, s) with the
        # diag j==i holding the masked partial sums (p pre-reduced on DVE)
        for bi in range(BLOC):
            for r2 in range(2):
                for q in range(4):
                    for i in range(WG):
                        c0 = (bi * 2 + r2) * 2 * WG + i * 2
                        det_total += float(od[32 * q + i, c0 : c0 + 2].sum())
        tag_total += float(r["out_tag"].sum(dtype=np.float64))

    det_mean = det_total / (B * S * N_PARTS * H * W)
    tag_mean = tag_total / (B * S)
    return np.float32(TAG_W * tag_mean + HM_W * det_mean)


# revision 43
# speedup vs baseline: 1.0771x; 1.0260x over previous
"""Trainium2 Bass kernel for nn_LossSupervisedTags (tag + heatmap MSE loss).

Contract: kernel(**inputs) takes the FULL unsharded inputs (as produced by
setup_inputs) and returns the FULL scalar output.  Internally the batch dim
(B=32) is sharded 4-images-per-core across 8 NeuronCores; each core computes
its local tag / heatmap loss partial sums on device, and the host combines
the 8 partial sums into the final scalar mean.

Host staging: slices per-core shards, transposes dets/heat to [h, p, w] so
every DMA descriptor is a fat contiguous run, and gathers the 510 predicted
tags per image (index staging for the tag loss).

Per-core device pipeline:
  DVE   : diff = dets - heat on 2-stack PAIR slabs [h=128, (s,p,w)=4352]
          (heat broadcast via 0-stride AP; bf16 2x) in pure arrival order —
          DVE's queue is kept free of anything that could park it on a
          slower engine's semaphore (measured ~2-4us bubbles otherwise).
  ACT   : squares (table Square) for images 0-2 — ~75% duty with
          arrival-paced gaps; ~6us of *sustained simultaneous* DVE+ACT
          density trips an activity throttle that clamps the DMA fabric
          ~4x (and deeper states also downclock engines ~20%).
  PE    : per (image, stack-pair) matmul rounds of 16 matmuls, each
          covering 8 w-columns: lhsT = mask 8-col slice, rhs = sq viewed
          [h, w8, s, p]; group g targets PE column quadrant g%4
          (tile_position) so LDWEIGHTS+MATMUL pairs pipeline in different
          sub-arrays.  Each image owns a 2-bank [128,1024] f32 PSUM tile
          (region s01 at [0:272], s23 at [512:784]) — matmul accumulation
          never crosses a 2KB bank.  PE runs at its low p-state
          (~2.8ns/col), so rounds are split fine-grained and the two
          closing rounds are single-stack (8 matmuls' worth of cols).
  DVE   : per-region reduce over parts (axis X) into a [128,128] f32
          accumulator — the out DMA then moves only 65KB.
  tag   : gathered pred tags packed [128,64]; (pt-gt)^2*vis reduced on the
          POOL engine (gpsimd) with its out_tag DMA on SWDGE, so the slow
          scalar-ring tagin transfer never blocks DVE or ACT.

DMA plan (sync HWDGE ring, one tile per transfer): heat0, image-0/1 pairs,
heat1, image-3's two SINGLE stacks mid-stream (their chains fill DVE's
arrival gaps and the throttle-exposed stream tail then carries pairs that
are mostly landed), image-2 pairs, image-3's s01 pair last (its chain is
the shortest tail).  mask/tagin ride the scalar HWDGE ring.

Endgame: image 2's s23 squares run as two ACT singles with per-stack
rounds, image 3's s01 squares as two DVE singles with per-stack rounds;
deferred rounds/reduces are emitted last, ordered by dependency readiness
(the static scheduler re-orders with an optimistic PE model — emission
position is the only reliable control).
"""

import sys
import types

import ml_dtypes
import numpy as np

import concourse.bacc as bacc
import concourse.mybir as mybir
from concourse.tile import TileContext
from concourse.bass_utils import run_bass_kernel_spmd

# If BASS_TRACE is set in the environment but this image lacks
# antenv.axon_hooks, run_bass_kernel_spmd would die on import; register a
# no-op hook module so tracing degrades gracefully instead.
try:
    import antenv.axon_hooks  # noqa: F401
except ImportError:
    try:
        import antenv

        _m = types.ModuleType("antenv.axon_hooks")
        _m.get_axon_ntff_profile_hook = lambda: None
        _m.set_axon_ntff_profile_hook = lambda h: None
        sys.modules["antenv.axon_hooks"] = _m
        antenv.axon_hooks = _m
    except ImportError:
        pass

# Problem constants (hardcoded per harness contract)
B, S, C, H, W = 32, 4, 34, 128, 128
N_PARTS, TAG_DIM, M = 17, 1, 30
TAG_W, HM_W = 0.001, 1.0
NCORES = 8
BLOC = B // NCORES            # 4 images per core
FREE = N_PARTS * W            # 2176 free elems per (b, s) tile
KP = M * N_PARTS              # 510 keypoints per image
KP_COLS = 4                   # ceil(510 / 128) columns per (b, s)
TAG_COLS = BLOC * S * KP_COLS  # 64
WG = 8                        # w-columns per matmul group
NG = W // WG                  # 16 matmul groups per round
RSQ = WG * 2 * N_PARTS        # 272 psum cols per (image, stack-pair) region
PSTRIDE = 512                 # psum region stride (bank-aligned, f32 elems)

_cache = {}

# Elementwise plan.  Subs all ride DVE (only engine with 2-tensor bf16 2x)
# in pure arrival order — anything interleaved into DVE's queue delays later
# subs through the scheduler's quantized cross-engine waits.  ACT owns all
# six pair squares (kept ~75% duty, arrival-paced gaps dodge the DMA
# throttle); image 3's four squares land on DVE right after its subs (the
# stream is done by then, DVE is free, and the 1.3us/square latency beats
# ACT's 2.1 in the tail).


def _build():
    f32 = mybir.dt.float32
    bf16 = mybir.dt.bfloat16
    nc = bacc.Bacc(
        "TRN2", target_bir_lowering=False, debug=False, num_devices=NCORES
    )
    # Host pre-interleaves dets into 2-stack pairs [pair, H, (s p w)] and
    # heat into 2-image pairs [pair, H, (b p w)] so every bulk transfer is a
    # single contiguous 8.7KB run per partition (max DMA-engine efficiency);
    # image 3's last two stacks stay as single-stack transfers [H, (p w)]
    # for fine-grained tail chasing.
    dets_p = nc.dram_tensor(
        "dets_p", [7, H, 2 * FREE], bf16, kind="ExternalInput"
    )
    dets_s = nc.dram_tensor("dets_s", [2, H, FREE], bf16, kind="ExternalInput")
    heat_p = nc.dram_tensor("heat_p", [2, H, 2 * FREE], bf16, kind="ExternalInput")
    maskw = nc.dram_tensor("maskw", [H, BLOC * W], bf16, kind="ExternalInput")
    tagin = nc.dram_tensor("tagin", [128, 3 * TAG_COLS], f32, kind="ExternalInput")
    out_det = nc.dram_tensor(
        "out_det", [128, BLOC * 2 * 2 * WG], f32, kind="ExternalOutput"
    )
    out_tag = nc.dram_tensor("out_tag", [1, 1], f32, kind="ExternalOutput")

    with TileContext(nc) as tc:
        with (
            tc.tile_pool(name="const", bufs=1) as cpool,
            tc.tile_pool(name="heatp", bufs=2) as hpool,
            tc.tile_pool(name="detp", bufs=7) as dpool,
            tc.tile_pool(name="dets3", bufs=2) as spool,
            tc.tile_pool(name="diffp", bufs=3) as fpool,
            tc.tile_pool(name="diffs", bufs=2) as spool_f,
            tc.tile_pool(name="sqp", bufs=4) as qpool,
            tc.tile_pool(name="psum0", bufs=1, space="PSUM") as ppool0,
            tc.tile_pool(name="psum1", bufs=1, space="PSUM") as ppool1,
            tc.tile_pool(name="psum2", bufs=1, space="PSUM") as ppool2,
            tc.tile_pool(name="psum3", bufs=1, space="PSUM") as ppool3,
        ):
            # Small inputs ride the scalar HWDGE ring so they don't queue
            # behind the bulk det stream on the sync ring.
            mask_t = cpool.tile([128, BLOC * W], bf16)
            nc.scalar.dma_start(out=mask_t[:], in_=maskw[:])
            tag_t = cpool.tile([128, 3 * TAG_COLS], f32)
            nc.scalar.dma_start(out=tag_t[:], in_=tagin[:])

            # Dummy activation so the Square table set loads during the DMA
            # ramp instead of delaying the first real square.
            warm_t = cpool.tile([1, 8], f32)
            warm_o = cpool.tile([1, 8], f32)
            nc.gpsimd.memset(warm_t[:], 0.0)
            nc.scalar.activation(
                warm_o[:], warm_t[:], mybir.ActivationFunctionType.Square
            )

            # 11 transfers up-front on the sync ring in need-order, one tile
            # per transfer.  Measured facts behind this shape: (a) single
            # contiguous runs per partition are the efficient DMA unit (the
            # host interleave makes pairs one 8.7KB run); (b) multiple
            # outstanding DMAs into ONE tile serialize on that tile's
            # semaphore; (c) the dynamic-HWDGE path keeps ~12 transfers in
            # flight and stalls the issuing sequencer when full — slots free
            # progressively, so the last two (needed last) still arrive in
            # time.  Within a ring, descriptors drain FIFO.
            # Issue order: image 0/1 pairs, heat1, then image 3's two singles
            # MID-stream (their chains fill DVE's natural gaps, and the
            # stream tail — where the activity throttle may clip DMA — then
            # carries pairs that are mostly landed before it fires), then
            # image 2's pairs and image 3's s01 pair last.
            heat_tiles = {}
            det_tiles = {}
            pair_tiles = {}

            def emit_heat(b0):
                hp = hpool.tile([128, 2 * FREE], bf16, name="heat_t", tag="heat_t")
                nc.sync.dma_start(out=hp[:], in_=heat_p[b0 // 2])
                heat_tiles[b0] = hp[:, 0:FREE]
                heat_tiles[b0 + 1] = hp[:, FREE : 2 * FREE]

            def emit_pair(b, s0, pair_idx):
                dp = dpool.tile([128, 2 * FREE], bf16, name="det_t", tag="det_t")
                nc.sync.dma_start(out=dp[:], in_=dets_p[pair_idx])
                pair_tiles[(b, s0)] = dp[:]
                det_tiles[(b, s0)] = dp[:, 0:FREE]
                det_tiles[(b, s0 + 1)] = dp[:, FREE : 2 * FREE]

            emit_heat(0)
            emit_pair(0, 0, 0)
            emit_pair(0, 2, 1)
            emit_pair(1, 0, 2)
            emit_pair(1, 2, 3)
            emit_heat(2)
            for s in (2, 3):
                ds = spool.tile([128, FREE], bf16, name="det_s", tag="det_s")
                nc.sync.dma_start(out=ds[:], in_=dets_s[s - 2])
                det_tiles[(BLOC - 1, s)] = ds[:]
            emit_pair(2, 0, 4)
            emit_pair(2, 2, 5)
            emit_pair(3, 0, 6)

            acc_det = cpool.tile([128, BLOC * 2 * 2 * WG], f32)

            def tag_block():
                # Tag loss (tiny) on the POOL engine: gated only by the
                # slow scalar-ring tagin transfer, which Pool can wait on
                # without holding up the DVE det pipeline.  The out_tag DMA
                # issues from Pool (SWDGE) so its tag-semaphore wait can
                # never park the ACT or sync sequencers.
                ptg_t = tag_t[:, 0:TAG_COLS]
                gtv_t = tag_t[:, TAG_COLS : 2 * TAG_COLS]
                vis_t = tag_t[:, 2 * TAG_COLS : 3 * TAG_COLS]
                e_t = cpool.tile([128, TAG_COLS], f32)
                ev_t = cpool.tile([128, TAG_COLS], f32)
                scr_t = cpool.tile([128, TAG_COLS], f32)
                tag_acc = cpool.tile([1, 1], f32)
                nc.gpsimd.tensor_sub(e_t[:], ptg_t, gtv_t)
                nc.gpsimd.tensor_mul(ev_t[:], e_t[:], vis_t)
                nc.gpsimd.tensor_mul(scr_t[:], e_t[:], ev_t[:])
                nc.gpsimd.tensor_reduce(
                    tag_acc[:], scr_t[:],
                    axis=mybir.AxisListType.XYZWC, op=mybir.AluOpType.add,
                )
                nc.gpsimd.dma_start(out=out_tag[:], in_=tag_acc[:])

            # ---- heatmap (det) loss ----
            # Each image owns a 2-bank [128, 1024] f32 PSUM tile: the s01
            # round accumulates in region [0:272] (bank 0), s23 in [512:784]
            # (bank 1) — per-matmul accumulation never crosses a 2KB bank.
            # 16 matmuls per round, each covering 8 w-columns: lhsT = mask
            # 8-col slice, rhs = sq viewed [h, w8, s, p].  Group g targets
            # PE column quadrant g % 4 (tile_position) so consecutive
            # LDWEIGHTS+MATMUL pairs pipeline in different sub-arrays; the
            # 4 groups of a quadrant accumulate.  Useful outputs:
            # psum[32q+j, region + j-th 34-block].
            def emit_round(b, psum_t, sq_g, s0, s1):
                region = 0 if s0 < 2 else PSTRIDE
                psum_r = psum_t[:, region : region + RSQ].rearrange(
                    "m (j s p) -> m j s p", j=WG, s=2
                )
                srel0, srel1 = s0 % 2, (s1 - 1) % 2 + 1
                for g in range(NG):
                    q = g % 4
                    nc.tensor.matmul(
                        psum_r[32 * q : 32 * q + WG, :, srel0:srel1, :],
                        lhsT=mask_t[:, b * W + WG * g : b * W + WG * (g + 1)],
                        rhs=sq_g[:, WG * g : WG * (g + 1), s0:s1, :],
                        start=(g < 4),
                        stop=(g >= NG - 4),
                        tile_position=(0, 32 * q),
                    )

            ppools = [ppool0, ppool1, ppool2, ppool3]
            psum_ts = {}
            sq_ts = {}
            sq_gs = {}

            def img_state(b):
                sq_t = qpool.tile([128, S * FREE], bf16, name=f"sq{b}", tag="sq")
                psum_t = ppools[b].tile(
                    [128, 2 * PSTRIDE], f32, name=f"ps{b}", tag=f"ps{b}"
                )
                psum_ts[b] = psum_t
                sq_ts[b] = sq_t
                sq_gs[b] = sq_t[:].rearrange(
                    "q (s p w) -> q w s p", s=S, p=N_PARTS
                )

            def pair_unit(b, s0, sq_eng="act", round_=True, split_sq=False):
                # one sub + one square per 2-stack slab (heat broadcast)
                heat_b = heat_tiles[b].unsqueeze(1).broadcast_to([128, 2, FREE])
                diff_t = fpool.tile(
                    [128, 2 * FREE], bf16, name="diffp_t", tag="diffp_t"
                )
                src3 = pair_tiles[(b, s0)].rearrange("p (s f) -> p s f", s=2)
                dst3 = diff_t[:].rearrange("p (s f) -> p s f", s=2)
                nc.vector.tensor_sub(dst3, src3, heat_b)
                if split_sq:
                    for ds in (0, 1):
                        s = s0 + ds
                        nc.vector.tensor_mul(
                            sq_ts[b][:, s * FREE : (s + 1) * FREE],
                            diff_t[:, ds * FREE : (ds + 1) * FREE],
                            diff_t[:, ds * FREE : (ds + 1) * FREE],
                        )
                        emit_round(b, psum_ts[b], sq_gs[b], s, s + 1)
                    return
                sqdst = sq_ts[b][:, s0 * FREE : (s0 + 2) * FREE]
                if sq_eng == "act":
                    nc.scalar.activation(
                        sqdst, diff_t[:], mybir.ActivationFunctionType.Square
                    )
                else:
                    nc.vector.tensor_mul(sqdst, diff_t[:], diff_t[:])
                if round_:
                    emit_round(b, psum_ts[b], sq_gs[b], s0, s0 + 2)

            def single_sub(b, s):
                diff_t = spool_f.tile(
                    [128, FREE], bf16, name="diffs_t", tag="diffs_t"
                )
                nc.vector.tensor_sub(diff_t[:], det_tiles[(b, s)], heat_tiles[b])
                return diff_t

            def single_sq(b, s, diff_t):
                dst = sq_ts[b][:, s * FREE : (s + 1) * FREE]
                nc.vector.tensor_mul(dst, diff_t[:], diff_t[:])
                emit_round(b, psum_ts[b], sq_gs[b], s, s + 1)

            for b in range(BLOC):
                img_state(b)
            # Images 0-1: pairs, ACT squares, rounds inline.
            pair_unit(0, 0)
            pair_unit(0, 2)
            tag_block()
            pair_unit(1, 0)
            pair_unit(1, 2)
            # Image 3's s2/s3 arrive mid-stream: their whole chains fill
            # DVE's arrival gaps and retire PSUM bank-3's B region early.
            d32 = single_sub(3, 2)
            single_sq(3, 2, d32)
            d33 = single_sub(3, 3)
            single_sq(3, 3, d33)
            # Image 2: ACT squares.  The s23 slab squares as two singles
            # (2.1us grains instead of one 3.9) with per-stack rounds, so
            # PE's last image-2 round starts and ends earlier; rounds are
            # deferred past image 3's so they cannot block it in PE's
            # static order.
            pair_unit(2, 0)
            d22 = pair_tiles[(2, 2)]
            diff22 = fpool.tile([128, 2 * FREE], bf16, name="diffp_t", tag="diffp_t")
            heat_b2 = heat_tiles[2].unsqueeze(1).broadcast_to([128, 2, FREE])
            nc.vector.tensor_sub(
                diff22[:].rearrange("p (s f) -> p s f", s=2),
                d22.rearrange("p (s f) -> p s f", s=2),
                heat_b2,
            )
            for s in (2, 3):
                nc.scalar.activation(
                    sq_ts[2][:, s * FREE : (s + 1) * FREE],
                    diff22[:, (s - 2) * FREE : (s - 1) * FREE],
                    mybir.ActivationFunctionType.Square,
                )
            # Image 3's s01 pair closes the stream: one sub, two DVE single
            # squares + per-stack rounds (shortest closing PE grains).
            pair_unit(3, 0, sq_eng="dve", round_=False, split_sq=True)
            emit_round(2, psum_ts[2], sq_gs[2], 2, 3)
            emit_round(2, psum_ts[2], sq_gs[2], 3, 4)

            # Per-region p-axis reductions (DVE, ~0.45us each) emitted LAST:
            # each waits on its region's final matmul; the out DMA then
            # carries only [128, 128] f32.  Latest-dependency regions go
            # last so the static scheduler cannot park DVE on them.
            def reduce_region(b, region, off):
                src = psum_ts[b][:, region * PSTRIDE : region * PSTRIDE + RSQ]
                dst = acc_det[:, off : off + 2 * WG].rearrange(
                    "m (j s) -> m j s", j=WG
                )
                nc.vector.reduce_sum(
                    dst, src.rearrange("m (j s p) -> m j s p", j=WG, s=2),
                    axis=mybir.AxisListType.X,
                )

            order = [(0, 0), (0, 1), (1, 0), (1, 1), (3, 1), (2, 0), (3, 0), (2, 1)]
            for b, region in order:
                reduce_region(b, region, (b * 2 + region) * 2 * WG)

            # Output DMA issues from the sync sequencer, which is parked on
            # its wait by then and fires as soon as the last reduce lands.
            nc.sync.dma_start(out=out_det[:], in_=acc_det[:])
    nc.compile()
    return nc


def _pack(vals):
    """vals: (BLOC, S, KP) float32 -> [128, TAG_COLS] with col = b*16+s*4+j,
    partition k holding element j*128+k of the zero-padded 512 vector."""
    padded = np.zeros((BLOC, S, KP_COLS * 128), np.float32)
    padded[..., :KP] = vals
    return (
        padded.reshape(BLOC, S, KP_COLS, 128)
        .transpose(3, 0, 1, 2)
        .reshape(128, TAG_COLS)
    )


def kernel(preds, masks, keypoints_idx, keypoints_vis, gt_tags, heatmaps):
    preds = np.asarray(preds, dtype=np.float32)
    masks = np.asarray(masks, dtype=np.float32)
    keypoints_idx = np.asarray(keypoints_idx)
    keypoints_vis = np.asarray(keypoints_vis, dtype=np.float32)
    gt_tags = np.asarray(gt_tags, dtype=np.float32)
    heatmaps = np.asarray(heatmaps, dtype=np.float32)

    if "nc" not in _cache:
        _cache["nc"] = _build()
    nc = _cache["nc"]

    # Host-side input staging: gather predicted tags at keypoint locations
    # (index-based staging; all loss arithmetic runs on device).
    tags = preds[:, :, N_PARTS:].reshape(B, S, N_PARTS * H * W)
    flat_idx = keypoints_idx.reshape(B, 1, KP).astype(np.int64)
    pt = np.take_along_axis(tags, np.broadcast_to(flat_idx, (B, S, KP)), axis=2)
    gt = gt_tags.reshape(B, KP)
    vi = keypoints_vis.reshape(B, KP)

    in_maps = []
    for c in range(NCORES):
        b0 = c * BLOC
        sl = slice(b0, b0 + BLOC)
        tag_in = np.concatenate(
            [
                _pack(pt[sl]),
                _pack(np.broadcast_to(gt[sl][:, None, :], (BLOC, S, KP))),
                _pack(np.broadcast_to(vi[sl][:, None, :], (BLOC, S, KP))),
            ],
            axis=1,
        )
        # [BLOC, S, 17, H, W] -> [BLOC, S, H, 17, W], then pack 2-stack
        # pairs as [H, (s p w)] and heat 2-image pairs as [H, (b p w)] so
        # each bulk DMA is one contiguous run per partition.
        dt_ = preds[sl, :, :N_PARTS].transpose(0, 1, 3, 2, 4)
        ht_ = heatmaps[sl].transpose(0, 2, 1, 3)
        dets_p = np.stack(
            [
                dt_[b, s0 : s0 + 2].transpose(1, 0, 2, 3).reshape(H, 2 * FREE)
                for b, s0 in (
                    (0, 0), (0, 2), (1, 0), (1, 2), (2, 0), (2, 2), (3, 0),
                )
            ]
        )
        dets_s = dt_[3, 2:4].reshape(2, H, FREE)
        heat_pk = np.stack(
            [
                ht_[b0 : b0 + 2].transpose(1, 0, 2, 3).reshape(H, 2 * FREE)
                for b0 in (0, 2)
            ]
        )
        in_maps.append(
            {
                "dets_p": np.ascontiguousarray(dets_p).astype(ml_dtypes.bfloat16),
                "dets_s": np.ascontiguousarray(dets_s).astype(ml_dtypes.bfloat16),
                "heat_p": np.ascontiguousarray(heat_pk).astype(ml_dtypes.bfloat16),
                # [BLOC, H, W] -> [H, BLOC*W]
                "maskw": np.ascontiguousarray(
                    masks[sl].transpose(1, 0, 2).reshape(H, BLOC * W)
                ).astype(ml_dtypes.bfloat16),
                "tagin": np.ascontiguousarray(tag_in),
            }
        )

    res = run_bass_kernel_spmd(nc, in_maps, list(range(NCORES)))
    _cache["last_results"] = res

    det_total = 0.0
    tag_total = 0.0
    for r in res.results:
        od = r["out_det"].astype(np.float64)
        # image b, region r2 (stack pair): row 32q+i, cols (j, s) with the
        # diag j==i holding the masked partial sums (p pre-reduced on DVE)
        for bi in range(BLOC):
            for r2 in range(2):
                for q in range(4):
                    for i in range(WG):
                        c0 = (bi * 2 + r2) * 2 * WG + i * 2
                        det_total += float(od[32 * q + i, c0 : c0 + 2].sum())
        tag_total += float(r["out_tag"].sum(dtype=np.float64))

    det_mean = det_total / (B * S * N_PARTS * H * W)
    tag_mean = tag_total / (B * S)
    return np.float32(TAG_W * tag_mean + HM_W * det_mean)
